# revision 1
# baseline (speedup 1.0000x reference)
"""HANConv Trainium2 kernel (8 NeuronCores, SPMD, full-I/O contract).

Strategy (v2)
-------------
Destination-sharded, fully core-independent:
  * Each core owns 1/8 of destination nodes for BOTH relations
    (writes: author->paper, written: paper->author).
  * Edges are sorted by (dst window, src half, src) on host. Per window,
    source rows are gathered as fp8(e4m3) 256B rows via gpsimd.dma_gather,
    round-robin over 4 SWDGE queues (4x the single-queue descriptor
    throughput; the gather is descriptor-bound, so fp8 also halves bytes),
    and segment-summed with fp8 one-hot matmuls accumulating in f32 PSUM.
  * Aggregating RAW features (M = A @ x, then per-dst 1/deg scale on the
    scalar engine) lets every later transform be a dense matmul from M with
    host-folded weights, so no cross-core exchange is ever needed.
  * 2-candidate semantic softmax is rewritten tanh-only:
        out = p + tanh(0.5*(s_h - s_agg)) * q
        p = 0.5*(h + agg),  q = 0.5*(h - agg)
    with the 0.5 factors folded into the weights on host. The scalar
    engine therefore never switches activation tables.
  * Scores use one fused DVE tensor_tensor_reduce:
        dsc = 0.5 * sum(w_score * (tanh(z_h) - tanh(z_agg)))
  * Self path computed from host-transposed x slices with folded weights.
  * Outputs written bf16 and upcast to f32 on host.
"""

import sys

sys.path.insert(0, "/opt/trn_rl_repo")

import numpy as np
import ml_dtypes

import concourse.bacc as bacc
import concourse.mybir as mybir
import concourse.tile as tile
from concourse.bass_utils import run_bass_kernel_spmd

P = 128
N = 50000
D = 256
HALF = 32768  # int16 gather index limit
NCORES = 8
NW_TOTAL = (N + P - 1) // P            # 391 destination windows
NWIN = (NW_TOTAL + NCORES - 1) // NCORES  # 49 windows per core
NW_ALLOC = NWIN * NCORES               # 392 (incl. 1 phantom window)
NPAD = NWIN * P                        # 6272 output rows per core

BF16 = ml_dtypes.bfloat16
FP8 = ml_dtypes.float8_e4m3
F32 = np.float32

USE_FP8 = True
NQ = 4


def _pairs():
    """Window slots grouped into gather pairs: [(0,1), (2,3), ..., (48,)]."""
    out = []
    w = 0
    while w < NWIN:
        out.append((w, w + 1) if w + 1 < NWIN else (w,))
        w += 2
    return out


# ---------------------------------------------------------------- host prep
def _prep_relation(row, col):
    """Sort edges by (dst window, src half, src); per-slot dynamic widths.

    Slot widths c_lo/c_hi[w] are the max over the 8 cores so the SPMD
    program is common. Gathers are issued per window PAIR (lo and hi
    halves separately) so the idx layout per core is, in pair order:
      [pair lo: slots w0|w1 ...][pair hi: slots w0|w1 ...] ...
    colf layout per core is per-slot: [slot: lo blocks | hi blocks] ...

    Returns (idx16_percore [NCORES,16,total8], colf_percore
    [NCORES,P,total_call], recip [P,NW_ALLOC], c_lo[NWIN], c_hi[NWIN]).
    """
    E = row.shape[0]
    key = (col // P) * 2 + (row >= HALF)
    order = np.lexsort((row, key))
    ks = key[order]
    rs = row[order].astype(np.int64)
    cs = col[order].astype(np.int64)

    counts = np.bincount(key, minlength=NW_ALLOC * 2).astype(np.int64)
    lo_cnt = counts[0::2].reshape(NCORES, NWIN)
    hi_cnt = counts[1::2].reshape(NCORES, NWIN)
    c_lo = np.maximum(1, -(-lo_cnt.max(axis=0) // P))  # [NWIN]
    c_hi = np.maximum(1, -(-hi_cnt.max(axis=0) // P))  # [NWIN]
    call_w = c_lo + c_hi
    off = np.zeros(NWIN + 1, dtype=np.int64)
    off[1:] = np.cumsum(call_w)
    total_call = int(off[-1])

    # idx layout offsets (in index units) per (slot, half), pair-ordered
    idx_base = np.zeros((NWIN, 2), dtype=np.int64)
    pos = 0
    for pr_ in _pairs():
        for w in pr_:
            idx_base[w, 0] = pos
            pos += int(c_lo[w]) * P
        for w in pr_:
            idx_base[w, 1] = pos
            pos += int(c_hi[w]) * P
    total_idx = pos

    grp_start = np.zeros(NW_ALLOC * 2 + 1, dtype=np.int64)
    np.cumsum(counts, out=grp_start[1:])
    rank = np.arange(E, dtype=np.int64) - grp_start[ks]
    w_of = ks // 2
    core = w_of // NWIN
    slot = w_of % NWIN
    hi_of = ks % 2

    idx_flat = np.zeros(NCORES * total_idx, dtype=np.int16)
    ipos = core * total_idx + idx_base[slot, hi_of] + rank
    idx_flat[ipos] = (rs - HALF * hi_of).astype(np.int16)
    col_flat = np.full(NCORES * total_call * P, -1.0, dtype=F32)
    cpos = core * (total_call * P) + (off[slot] + hi_of * c_lo[slot]) * P + rank
    col_flat[cpos] = (cs - w_of * P).astype(F32)

    # wrap idx per gather region: region r of length L -> [16, L*8/16...]
    idx_pc = idx_flat.reshape(NCORES, total_idx)
    parts = []
    pos = 0
    for pr_ in _pairs():
        for half, carr in ((0, c_lo), (1, c_hi)):
            L = int(sum(carr[w] for w in pr_)) * P
            reg = idx_pc[:, pos: pos + L]
            parts.append(reg.reshape(NCORES, L // 16, 16).transpose(0, 2, 1))
            pos += L
    idx16 = np.concatenate(parts, axis=2)  # [NCORES, 16, total_idx//16]

    colf = col_flat.reshape(NCORES, total_call, P).transpose(0, 2, 1)

    deg = np.bincount(col, minlength=NW_ALLOC * P).astype(F32)[: NW_ALLOC * P]
    recip = (1.0 / np.maximum(deg, 1.0)).reshape(NW_ALLOC, P).T  # [P, NW]
    return idx16, colf, recip, tuple(int(x) for x in c_lo), tuple(int(x) for x in c_hi)


def _host_prep(inp):
    pr = {}
    pr["wr"] = _prep_relation(np.asarray(inp["row_writes"]), np.asarray(inp["col_writes"]))
    pr["wn"] = _prep_relation(np.asarray(inp["row_written"]), np.asarray(inp["col_written"]))

    xa = np.asarray(inp["x_author"], dtype=F32)
    xp = np.asarray(inp["x_paper"], dtype=F32)
    if USE_FP8:
        # fp8 gather tables, viewed as bf16 [N, 128] for the byte-moving gather
        pr["xa8"] = xa.astype(FP8).view(np.uint16).view(BF16)
        pr["xp8"] = xp.astype(FP8).view(np.uint16).view(BF16)
    else:
        pr["xa8"] = xa.astype(BF16)
        pr["xp8"] = xp.astype(BF16)

    # per-core transposed x slices (for the self path of the dst shard)
    xta, xtp = [], []
    for c in range(NCORES):
        r0, r1 = c * NPAD, min(N, (c + 1) * NPAD)
        sa = np.zeros((D, NPAD), dtype=BF16)
        sp = np.zeros((D, NPAD), dtype=BF16)
        sa[:, : r1 - r0] = xa[r0:r1].T
        sp[:, : r1 - r0] = xp[r0:r1].T
        xta.append(sa)
        xtp.append(sp)
    pr["xta"], pr["xtp"] = xta, xtp

    W_sem = np.asarray(inp["W_sem"], dtype=F32)
    b_sem = np.asarray(inp["b_sem"], dtype=F32)
    w_score = np.asarray(inp["w_score"], dtype=F32)

    def w(name):
        return np.asarray(inp[name], dtype=F32)

    # folded weights per relation: (dst self weight, rel weight)
    for tag, wself, bself, wrel in (
        ("wr", w("W_self_paper"), w("b_self_paper"), w("W_rel_writes")),
        ("wn", w("W_self_author"), w("b_self_author"), w("W_rel_written")),
    ):
        pr[f"wp_self_{tag}"] = (0.5 * wself).astype(BF16)
        pr[f"wp_rel_{tag}"] = (0.5 * wrel).astype(BF16)
        pr[f"wq_rel_{tag}"] = (-0.5 * wrel).astype(BF16)
        pr[f"wf_self_{tag}"] = (wself @ W_sem).astype(BF16)
        pr[f"wf_rel_{tag}"] = (wrel @ W_sem).astype(BF16)
        # bias rows: [1, 3*D] = (0.5*b_self | b_self@W_sem + b_sem | b_sem)
        pr[f"brows_{tag}"] = np.concatenate([
            0.5 * bself, bself @ W_sem + b_sem, b_sem,
        ]).reshape(1, 3 * D).astype(BF16)

    # pre-scaled by the 0.5 from sigmoid(x) = 0.5*(1+tanh(x/2))
    pr["wrep"] = np.tile(0.5 * w_score, (P, 1)).astype(F32)
    pr["iota"] = np.tile(np.arange(P, dtype=F32), (P, 1)).astype(BF16)
    pr["ident"] = np.eye(P, dtype=F32).astype(BF16)
    pr["ones"] = np.ones((1, P), dtype=BF16)
    return pr


# ---------------------------------------------------------------- program
def build_program(nwin, c_lo_wr, c_hi_wr, c_lo_wn, c_hi_wn, scale=1,
                  nq=4, use_fp8=True, skip_gather=False, gather_only=False):
    f32 = mybir.dt.float32
    bf16 = mybir.dt.bfloat16
    f8 = mybir.dt.float8e4 if use_fp8 else mybir.dt.bfloat16
    i16 = mybir.dt.int16
    AF = mybir.ActivationFunctionType
    OP = mybir.AluOpType

    npad = nwin * P

    def pairs_of(n):
        out = []
        w = 0
        while w < n:
            out.append((w, w + 1) if w + 1 < n else (w,))
            w += 2
        return out

    wpairs = pairs_of(nwin)

    def rel_geom(c_lo, c_hi):
        call_w = [c_lo[w] + c_hi[w] for w in range(nwin)]
        off = [0]
        for w in range(nwin):
            off.append(off[-1] + call_w[w])
        total_call = off[-1]
        # pair-ordered idx offsets (in 16-wrapped columns, x8 replicas)
        pinfo = []
        pos = 0
        for pr_ in wpairs:
            wlo = sum(c_lo[w] for w in pr_)
            whi = sum(c_hi[w] for w in pr_)
            pinfo.append((pos, wlo, pos + 8 * wlo, whi))
            pos += 8 * (wlo + whi)
        return dict(call_w=call_w, off=off, total_call=total_call,
                    pinfo=pinfo, total8=pos,
                    maxlo=max(i[1] for i in pinfo),
                    maxhi=max(i[3] for i in pinfo),
                    maxcall=max(call_w))

    geom_wr = rel_geom(c_lo_wr, c_hi_wr)
    geom_wn = rel_geom(c_lo_wn, c_hi_wn)

    nc = bacc.Bacc("TRN2", target_bir_lowering=False, debug=False,
                   num_swdge_queues=nq)

    TW = P if use_fp8 else D
    xa8 = nc.dram_tensor("xa8", [N, TW], bf16, kind="ExternalInput")
    xp8 = nc.dram_tensor("xp8", [N, TW], bf16, kind="ExternalInput")
    xta = nc.dram_tensor("xta", [D, npad], bf16, kind="ExternalInput")
    xtp = nc.dram_tensor("xtp", [D, npad], bf16, kind="ExternalInput")

    wnames = []
    for tag in ("wr", "wn"):
        wnames += [f"wp_self_{tag}", f"wp_rel_{tag}", f"wq_rel_{tag}",
                   f"wf_self_{tag}", f"wf_rel_{tag}"]
    wdram = {n: nc.dram_tensor(n, [D, D], bf16, kind="ExternalInput") for n in wnames}
    brow_d = {tag: nc.dram_tensor(f"brows_{tag}", [1, 3 * D], bf16,
                                  kind="ExternalInput") for tag in ("wr", "wn")}
    wrep_d = nc.dram_tensor("wrep", [P, D], f32, kind="ExternalInput")
    iota_d = nc.dram_tensor("iota", [P, P], bf16, kind="ExternalInput")
    ident_d = nc.dram_tensor("ident", [P, P], bf16, kind="ExternalInput")
    ones_d = nc.dram_tensor("ones", [1, P], bf16, kind="ExternalInput")

    idx_wr_d = nc.dram_tensor("idx_wr", [P, geom_wr["total8"]], i16, kind="ExternalInput")
    idx_wn_d = nc.dram_tensor("idx_wn", [P, geom_wn["total8"]], i16, kind="ExternalInput")
    colf_wr_d = nc.dram_tensor("colf_wr", [P, geom_wr["total_call"]], bf16, kind="ExternalInput")
    colf_wn_d = nc.dram_tensor("colf_wn", [P, geom_wn["total_call"]], bf16, kind="ExternalInput")
    recip_wr_d = nc.dram_tensor("recip_wr", [P, nwin], f32, kind="ExternalInput")
    recip_wn_d = nc.dram_tensor("recip_wn", [P, nwin], f32, kind="ExternalInput")

    oa = nc.dram_tensor("oa", [npad, D], bf16, kind="ExternalOutput")
    op_ = nc.dram_tensor("op", [npad, D], bf16, kind="ExternalOutput")

    with tile.TileContext(nc) as tc:
        with tc.tile_pool(name="const", bufs=1) as cpool, \
             tc.tile_pool(name="gbuf", bufs=3) as gpool, \
             tc.tile_pool(name="oh", bufs=3) as ohpool, \
             tc.tile_pool(name="sb", bufs=3) as sbpool, \
             tc.tile_pool(name="mps", bufs=2, space="PSUM") as mpool, \
             tc.tile_pool(name="tps", bufs=1, space="PSUM") as tpool, \
             tc.tile_pool(name="dps", bufs=1, space="PSUM") as dpool:

            def load(dram, shape, dtype, tag):
                t = cpool.tile(shape, dtype, tag=tag)
                nc.sync.dma_start(t[:], dram)
                return t

            iota_t = load(iota_d[:], [P, P], bf16, "c_iota")
            ident_t = load(ident_d[:], [P, P], bf16, "c_ident")
            ones_t = load(ones_d[:], [1, P], bf16, "c_ones")
            wrep_t = load(wrep_d[:], [P, D], f32, "c_wrep")
            wt = {n: (load(wdram[n][0:P, :], [P, D], bf16, f"c_{n}0"),
                      load(wdram[n][P:D, :], [P, D], bf16, f"c_{n}1")) for n in wnames}
            brow = {tag: load(brow_d[tag][:], [1, 3 * D], bf16, f"c_br{tag}")
                    for tag in ("wr", "wn")}
            xta_t = (load(xta[0:P, :], [P, npad], bf16, "c_xta0"),
                     load(xta[P:D, :], [P, npad], bf16, "c_xta1"))
            xtp_t = (load(xtp[0:P, :], [P, npad], bf16, "c_xtp0"),
                     load(xtp[P:D, :], [P, npad], bf16, "c_xtp1"))
            idx_wr_t = load(idx_wr_d[:], [P, geom_wr["total8"]], i16, "c_idxwr")
            idx_wn_t = load(idx_wn_d[:], [P, geom_wn["total8"]], i16, "c_idxwn")
            colf_wr_t = load(colf_wr_d[:], [P, geom_wr["total_call"]], bf16, "c_colfwr")
            colf_wn_t = load(colf_wn_d[:], [P, geom_wn["total_call"]], bf16, "c_colfwn")
            recip_wr_t = load(recip_wr_d[:], [P, nwin], f32, "c_recipwr")
            recip_wn_t = load(recip_wn_d[:], [P, nwin], f32, "c_recipwn")

            rels = [
                dict(tag="wr", table=xa8, idx=idx_wr_t, colf=colf_wr_t,
                     recip=recip_wr_t, c_lo=c_lo_wr, c_hi=c_hi_wr,
                     geom=geom_wr, xt=xtp_t, q0=0, out=op_),
                dict(tag="wn", table=xp8, idx=idx_wn_t, colf=colf_wn_t,
                     recip=recip_wn_t, c_lo=c_lo_wn, c_hi=c_hi_wn,
                     geom=geom_wn, xt=xta_t, q0=2, out=oa),
            ]
            for r in rels:
                tag = r["tag"]
                r["wp_self"] = wt[f"wp_self_{tag}"]
                r["wp_rel"] = wt[f"wp_rel_{tag}"]
                r["wq_rel"] = wt[f"wq_rel_{tag}"]
                r["wf_self"] = wt[f"wf_self_{tag}"]
                r["wf_rel"] = wt[f"wf_rel_{tag}"]
                r["brow"] = brow[tag]

            def emit_pair_gather(ip, r):
                geom = r["geom"]
                io_lo, wlo, io_hi, whi = geom["pinfo"][ip]
                tag = r["tag"]
                g_lo = gpool.tile([P, geom["maxlo"], D], f8, tag=f"glo{tag}")
                g_hi = gpool.tile([P, geom["maxhi"], D], f8, tag=f"ghi{tag}")
                if not skip_gather:
                    out_lo = (g_lo.bitcast(bf16) if use_fp8 else g_lo)[:, 0:wlo, :]
                    nc.gpsimd.dma_gather(
                        out_lo, r["table"][:],
                        r["idx"][:, io_lo: io_lo + 8 * wlo],
                        wlo * P, wlo * P, TW, single_packet=False,
                        queue_num=r["q0"] % nq)
                    out_hi = (g_hi.bitcast(bf16) if use_fp8 else g_hi)[:, 0:whi, :]
                    nc.gpsimd.dma_gather(
                        out_hi, r["table"][HALF:, :],
                        r["idx"][:, io_hi: io_hi + 8 * whi],
                        whi * P, whi * P, TW, single_packet=False,
                        queue_num=(r["q0"] + 1) % nq)
                return g_lo, g_hi

            def emit_window(w, r, g_lo, g_hi, lo0, hi0):
                tag = r["tag"]
                geom = r["geom"]
                c_lo, c_hi = r["c_lo"][w], r["c_hi"][w]
                call = c_lo + c_hi
                co = geom["off"][w]

                oh = ohpool.tile([P, geom["maxcall"], P], f8, tag=f"oh{tag}")
                nc.vector.tensor_tensor(
                    out=oh[:, 0:call, :],
                    in0=r["colf"][:, co: co + call, None].to_broadcast([P, call, P]),
                    in1=iota_t[:, None, :].to_broadcast([P, call, P]),
                    op=OP.is_equal)

                m_ps = mpool.tile([P, D], f32, tag="m")
                for k in range(call):
                    rhs = (g_lo[:, lo0 + k, :] if k < c_lo
                           else g_hi[:, hi0 + k - c_lo, :])
                    nc.tensor.matmul(out=m_ps[:], lhsT=oh[:, k, :], rhs=rhs,
                                     start=(k == 0), stop=(k == call - 1))

                # deg-normalize on the scalar engine (per-dst 1/deg scale)
                m_sb = sbpool.tile([P, D], bf16, tag="m_sb")
                nc.scalar.activation(out=m_sb[:], in_=m_ps[:], func=AF.Copy,
                                     scale=r["recip"][:, w: w + 1])

                mt = []
                for h2 in range(2):
                    t_ps = tpool.tile([P, P], bf16, tag=f"t{h2}")
                    nc.tensor.transpose(out=t_ps[:],
                                        in_=m_sb[:, h2 * P: (h2 + 1) * P],
                                        identity=ident_t[:])
                    mt_sb = sbpool.tile([P, P], bf16, tag=f"mt{h2}")
                    nc.scalar.activation(out=mt_sb[:], in_=t_ps[:], func=AF.Copy)
                    mt.append(mt_sb)

                xsl0 = r["xt"][0][:, w * P: (w + 1) * P]
                xsl1 = r["xt"][1][:, w * P: (w + 1) * P]
                br = r["brow"]

                def dense(ps, parts, brow_slice):
                    for i, (lhsT, rhs) in enumerate(parts):
                        nc.tensor.matmul(out=ps, lhsT=lhsT, rhs=rhs,
                                         start=(i == 0), stop=False)
                    nc.tensor.matmul(out=ps, lhsT=ones_t[:], rhs=brow_slice,
                                     start=False, stop=True)
                    return ps

                pt = dpool.tile([P, D], f32, tag="p")
                qt = dpool.tile([P, D], f32, tag="q")
                zht = dpool.tile([P, D], f32, tag="zh")
                zat = dpool.tile([P, D], f32, tag="za")
                p_ps = dense(pt[:],
                             [(xsl0, r["wp_self"][0][:]), (xsl1, r["wp_self"][1][:]),
                              (mt[0][:], r["wp_rel"][0][:]), (mt[1][:], r["wp_rel"][1][:])],
                             br[:, 0:D])
                q_ps = dense(qt[:],
                             [(xsl0, r["wp_self"][0][:]), (xsl1, r["wp_self"][1][:]),
                              (mt[0][:], r["wq_rel"][0][:]), (mt[1][:], r["wq_rel"][1][:])],
                             br[:, 0:D])
                zh_ps = dense(zht[:],
                              [(xsl0, r["wf_self"][0][:]), (xsl1, r["wf_self"][1][:])],
                              br[:, D:2 * D])
                za_ps = dense(zat[:],
                              [(mt[0][:], r["wf_rel"][0][:]), (mt[1][:], r["wf_rel"][1][:])],
                              br[:, 2 * D:3 * D])

                th = sbpool.tile([P, D], f32, tag="th")
                nc.scalar.activation(out=th[:], in_=zh_ps, func=AF.Tanh)
                ta = sbpool.tile([P, D], f32, tag="ta")
                nc.scalar.activation(out=ta[:], in_=za_ps, func=AF.Tanh)

                v = sbpool.tile([P, D], f32, tag="v")
                nc.vector.tensor_tensor(out=v[:], in0=th[:], in1=ta[:],
                                        op=OP.subtract)
                vw = sbpool.tile([P, D], f32, tag="vw")
                nc.vector.tensor_tensor(out=vw[:], in0=v[:], in1=wrep_t[:],
                                        op=OP.mult)
                dsc = sbpool.tile([P, 1], f32, tag="dsc")
                nc.vector.tensor_reduce(out=dsc[:], in_=vw[:],
                                        axis=mybir.AxisListType.X, op=OP.add)

                t_sc = sbpool.tile([P, 1], f32, tag="tsc")
                nc.scalar.activation(out=t_sc[:], in_=dsc[:], func=AF.Tanh)

                wq = sbpool.tile([P, D], f32, tag="wq")
                nc.vector.tensor_scalar(out=wq[:], in0=q_ps,
                                        scalar1=t_sc[:, 0:1], scalar2=None,
                                        op0=OP.mult)
                outt = sbpool.tile([P, D], bf16, tag="outt")
                nc.vector.tensor_tensor(out=outt[:], in0=wq[:], in1=p_ps,
                                        op=OP.add)
                nc.sync.dma_start(r["out"][w * P: (w + 1) * P, :], outt[:])

            last_g = None
            for _s in range(scale):
                for ip, pr_ in enumerate(wpairs):
                    gt = {r["tag"]: emit_pair_gather(ip, r) for r in rels}
                    last_g = gt[rels[0]["tag"]][0]
                    if gather_only:
                        continue
                    for j, w in enumerate(pr_):
                        for r in rels:
                            g_lo, g_hi = gt[r["tag"]]
                            lo0 = sum(r["c_lo"][v] for v in pr_[:j])
                            hi0 = sum(r["c_hi"][v] for v in pr_[:j])
                            emit_window(w, r, g_lo, g_hi, lo0, hi0)
            if gather_only:
                fin = sbpool.tile([P, D], bf16, tag="fin")
                nc.vector.tensor_copy(out=fin[:], in_=last_g[:, 0, :])
                nc.sync.dma_start(oa[0:P, :], fin[:])

    nc.compile()
    return nc


# ---------------------------------------------------------------- driver
_PROG_CACHE = {}


def _get_program(key):
    if key not in _PROG_CACHE:
        _PROG_CACHE[key] = build_program(*key)
    return _PROG_CACHE[key]


def _make_in_maps(pr):
    shared = dict(
        xa8=pr["xa8"], xp8=pr["xp8"],
        iota=pr["iota"], ident=pr["ident"], ones=pr["ones"], wrep=pr["wrep"],
        brows_wr=pr["brows_wr"], brows_wn=pr["brows_wn"],
    )
    for tag in ("wr", "wn"):
        for nm in ("wp_self", "wp_rel", "wq_rel", "wf_self", "wf_rel"):
            shared[f"{nm}_{tag}"] = pr[f"{nm}_{tag}"]
    idx_wr, colf_wr, recip_wr, _, _ = pr["wr"]
    idx_wn, colf_wn, recip_wn, _, _ = pr["wn"]
    in_maps = []
    for c in range(NCORES):
        w0, w1 = c * NWIN, (c + 1) * NWIN
        m = dict(shared)
        m["xta"] = pr["xta"][c]
        m["xtp"] = pr["xtp"][c]
        m["idx_wr"] = np.ascontiguousarray(np.tile(idx_wr[c], (8, 1)))
        m["idx_wn"] = np.ascontiguousarray(np.tile(idx_wn[c], (8, 1)))
        m["colf_wr"] = np.ascontiguousarray(colf_wr[c]).astype(BF16)
        m["colf_wn"] = np.ascontiguousarray(colf_wn[c]).astype(BF16)
        m["recip_wr"] = np.ascontiguousarray(recip_wr[:, w0:w1])
        m["recip_wn"] = np.ascontiguousarray(recip_wn[:, w0:w1])
        in_maps.append(m)
    return in_maps


def run(trace=False, tmpdir=None, **inputs):
    pr = _host_prep(inputs)
    _, _, _, c_lo_wr, c_hi_wr = pr["wr"]
    _, _, _, c_lo_wn, c_hi_wn = pr["wn"]
    nc = _get_program((NWIN, c_lo_wr, c_hi_wr, c_lo_wn, c_hi_wn, 1, NQ, USE_FP8))
    in_maps = _make_in_maps(pr)
    res = run_bass_kernel_spmd(nc, in_maps, list(range(NCORES)),
                               trace=trace, tmpdir=tmpdir)
    oa = np.empty((N, D), dtype=F32)
    op = np.empty((N, D), dtype=F32)
    for c in range(NCORES):
        r0, r1 = c * NPAD, min(N, (c + 1) * NPAD)
        oa[r0:r1] = res.results[c]["oa"][: r1 - r0].astype(F32)
        op[r0:r1] = res.results[c]["op"][: r1 - r0].astype(F32)
    return (oa, op), res


def kernel(**inputs):
    (oa, op), _ = run(trace=False, **inputs)
    return (oa, op)



# revision 9
# speedup vs baseline: 5.9814x; 5.9814x over previous
"""HANConv Trainium2 kernel (8 NeuronCores, SPMD, full-I/O contract).

Strategy (v2)
-------------
Destination-sharded, fully core-independent:
  * Each core owns 1/8 of destination nodes for BOTH relations
    (writes: author->paper, written: paper->author).
  * Edges are sorted by (dst window, src half, src) on host. Per window,
    source rows are gathered as fp8(e4m3) 256B rows via gpsimd.dma_gather,
    round-robin over 4 SWDGE queues (4x the single-queue descriptor
    throughput; the gather is descriptor-bound, so fp8 also halves bytes),
    and segment-summed with fp8 one-hot matmuls accumulating in f32 PSUM.
  * Aggregating RAW features (M = A @ x, then per-dst 1/deg scale on the
    scalar engine) lets every later transform be a dense matmul from M with
    host-folded weights, so no cross-core exchange is ever needed.
  * 2-candidate semantic softmax is rewritten tanh-only:
        out = p + tanh(0.5*(s_h - s_agg)) * q
        p = 0.5*(h + agg),  q = 0.5*(h - agg)
    with the 0.5 factors folded into the weights on host. The scalar
    engine therefore never switches activation tables.
  * Scores use one fused DVE tensor_tensor_reduce:
        dsc = 0.5 * sum(w_score * (tanh(z_h) - tanh(z_agg)))
  * Self path computed from host-transposed x slices with folded weights.
  * Outputs written bf16 and upcast to f32 on host.
"""

import sys

sys.path.insert(0, "/opt/trn_rl_repo")

import numpy as np
import ml_dtypes

import concourse.bacc as bacc
import concourse.mybir as mybir
import concourse.tile as tile
from concourse.bass_utils import run_bass_kernel_spmd

P = 128
N = 50000
D = 256
HALF = 32768  # int16 gather index limit
NCORES = 8
NW_TOTAL = (N + P - 1) // P            # 391 destination windows
NWIN = (NW_TOTAL + NCORES - 1) // NCORES  # 49 windows per core
NW_ALLOC = NWIN * NCORES               # 392 (incl. 1 phantom window)
NPAD = NWIN * P                        # 6272 output rows per core

BF16 = ml_dtypes.bfloat16
FP8 = ml_dtypes.float8_e4m3
F32 = np.float32

USE_FP8 = True
NQ = 4


GROUP = 2


def _pairs(group=None):
    """Window slots grouped into gather groups of `group` windows."""
    g = GROUP if group is None else group
    out = []
    w = 0
    while w < NWIN:
        out.append(tuple(range(w, min(w + g, NWIN))))
        w += g
    return out


# ---------------------------------------------------------------- host prep
def _prep_relation(row, col):
    """Sort edges by (dst window, src half, src); per-slot dynamic widths.

    Slot widths c_lo/c_hi[w] are the max over the 8 cores so the SPMD
    program is common. Gathers are issued per window PAIR (lo and hi
    halves separately) so the idx layout per core is, in pair order:
      [pair lo: slots w0|w1 ...][pair hi: slots w0|w1 ...] ...
    colf layout per core is per-slot: [slot: lo blocks | hi blocks] ...

    Returns (idx16_percore [NCORES,16,total8], colf_percore
    [NCORES,P,total_call], recip [P,NW_ALLOC], c_lo[NWIN], c_hi[NWIN]).
    """
    E = row.shape[0]
    key = (col // P) * 2 + (row >= HALF)
    order = np.lexsort((row, key))
    ks = key[order]
    rs = row[order].astype(np.int64)
    cs = col[order].astype(np.int64)

    counts = np.bincount(key, minlength=NW_ALLOC * 2).astype(np.int64)
    lo_cnt = counts[0::2].reshape(NCORES, NWIN)
    hi_cnt = counts[1::2].reshape(NCORES, NWIN)
    c_lo = np.maximum(1, -(-lo_cnt.max(axis=0) // P))  # [NWIN]
    c_hi = np.maximum(1, -(-hi_cnt.max(axis=0) // P))  # [NWIN]
    call_w = c_lo + c_hi
    off = np.zeros(NWIN + 1, dtype=np.int64)
    off[1:] = np.cumsum(call_w)
    total_call = int(off[-1])

    # idx layout offsets (in index units) per (slot, half), pair-ordered
    idx_base = np.zeros((NWIN, 2), dtype=np.int64)
    pos = 0
    for pr_ in _pairs():
        for w in pr_:
            idx_base[w, 0] = pos
            pos += int(c_lo[w]) * P
        for w in pr_:
            idx_base[w, 1] = pos
            pos += int(c_hi[w]) * P
    total_idx = pos

    grp_start = np.zeros(NW_ALLOC * 2 + 1, dtype=np.int64)
    np.cumsum(counts, out=grp_start[1:])
    rank = np.arange(E, dtype=np.int64) - grp_start[ks]
    w_of = ks // 2
    core = w_of // NWIN
    slot = w_of % NWIN
    hi_of = ks % 2

    idx_flat = np.zeros(NCORES * total_idx, dtype=np.int16)
    ipos = core * total_idx + idx_base[slot, hi_of] + rank
    idx_flat[ipos] = (rs - HALF * hi_of).astype(np.int16)
    col_flat = np.full(NCORES * total_call * P, -1.0, dtype=F32)
    cpos = core * (total_call * P) + (off[slot] + hi_of * c_lo[slot]) * P + rank
    col_flat[cpos] = (cs - w_of * P).astype(F32)

    # wrap idx per gather region: region r of length L -> [16, L*8/16...]
    idx_pc = idx_flat.reshape(NCORES, total_idx)
    parts = []
    pos = 0
    for pr_ in _pairs():
        for half, carr in ((0, c_lo), (1, c_hi)):
            L = int(sum(carr[w] for w in pr_)) * P
            reg = idx_pc[:, pos: pos + L]
            parts.append(reg.reshape(NCORES, L // 16, 16).transpose(0, 2, 1))
            pos += L
    idx16 = np.concatenate(parts, axis=2)  # [NCORES, 16, total_idx//16]

    colf = col_flat.reshape(NCORES, total_call, P).transpose(0, 2, 1)

    deg = np.bincount(col, minlength=NW_ALLOC * P).astype(F32)[: NW_ALLOC * P]
    recip = (1.0 / np.maximum(deg, 1.0)).reshape(NW_ALLOC, P).T  # [P, NW]
    return idx16, colf, recip, tuple(int(x) for x in c_lo), tuple(int(x) for x in c_hi)


def _host_prep(inp):
    pr = {}
    pr["wr"] = _prep_relation(np.asarray(inp["row_writes"]), np.asarray(inp["col_writes"]))
    pr["wn"] = _prep_relation(np.asarray(inp["row_written"]), np.asarray(inp["col_written"]))

    xa = np.asarray(inp["x_author"], dtype=F32)
    xp = np.asarray(inp["x_paper"], dtype=F32)
    if USE_FP8:
        # fp8 gather tables, viewed as bf16 [N, 128] for the byte-moving gather
        pr["xa8"] = xa.astype(FP8).view(np.uint16).view(BF16)
        pr["xp8"] = xp.astype(FP8).view(np.uint16).view(BF16)
    else:
        pr["xa8"] = xa.astype(BF16)
        pr["xp8"] = xp.astype(BF16)

    # per-core transposed x slices (for the self path of the dst shard)
    xta, xtp = [], []
    for c in range(NCORES):
        r0, r1 = c * NPAD, min(N, (c + 1) * NPAD)
        sa = np.zeros((D, NPAD), dtype=BF16)
        sp = np.zeros((D, NPAD), dtype=BF16)
        sa[:, : r1 - r0] = xa[r0:r1].T
        sp[:, : r1 - r0] = xp[r0:r1].T
        xta.append(sa)
        xtp.append(sp)
    pr["xta"], pr["xtp"] = xta, xtp

    W_sem = np.asarray(inp["W_sem"], dtype=F32)
    b_sem = np.asarray(inp["b_sem"], dtype=F32)
    w_score = np.asarray(inp["w_score"], dtype=F32)

    def w(name):
        return np.asarray(inp[name], dtype=F32)

    # folded weights per relation: (dst self weight, rel weight)
    for tag, wself, bself, wrel in (
        ("wr", w("W_self_paper"), w("b_self_paper"), w("W_rel_writes")),
        ("wn", w("W_self_author"), w("b_self_author"), w("W_rel_written")),
    ):
        pr[f"wp_self_{tag}"] = (0.5 * wself).astype(BF16)
        pr[f"wp_rel_{tag}"] = (0.5 * wrel).astype(BF16)
        pr[f"wq_rel_{tag}"] = (-0.5 * wrel).astype(BF16)
        pr[f"wf_self_{tag}"] = (wself @ W_sem).astype(BF16)
        pr[f"wf_rel_{tag}"] = (wrel @ W_sem).astype(BF16)
        # bias rows: [1, 3*D] = (0.5*b_self | b_self@W_sem + b_sem | b_sem)
        pr[f"brows_{tag}"] = np.concatenate([
            0.5 * bself, bself @ W_sem + b_sem, b_sem,
        ]).reshape(1, 3 * D).astype(BF16)

    # pre-scaled by the 0.5 from sigmoid(x) = 0.5*(1+tanh(x/2))
    pr["wrep"] = np.tile(0.5 * w_score, (P, 1)).astype(F32)
    pr["iota"] = np.tile(np.arange(P, dtype=F32), (P, 1)).astype(BF16)
    pr["ident"] = np.eye(P, dtype=F32).astype(BF16)
    pr["ones"] = np.ones((1, P), dtype=BF16)
    return pr


# ---------------------------------------------------------------- program
def build_program(nwin, c_lo_wr, c_hi_wr, c_lo_wn, c_hi_wn, scale=1,
                  nq=4, use_fp8=True, skip_gather=False, gather_only=False,
                  group=None, single_packet=False, tiny_idx=False,
                  gbufs=3):
    f32 = mybir.dt.float32
    bf16 = mybir.dt.bfloat16
    f8 = mybir.dt.float8e4 if use_fp8 else mybir.dt.bfloat16
    i16 = mybir.dt.int16
    AF = mybir.ActivationFunctionType
    OP = mybir.AluOpType

    npad = nwin * P

    g_ = GROUP if group is None else group

    def pairs_of(n):
        out = []
        w = 0
        while w < n:
            out.append(tuple(range(w, min(w + g_, n))))
            w += g_
        return out

    wpairs = pairs_of(nwin)

    def rel_geom(c_lo, c_hi):
        call_w = [c_lo[w] + c_hi[w] for w in range(nwin)]
        off = [0]
        for w in range(nwin):
            off.append(off[-1] + call_w[w])
        total_call = off[-1]
        # pair-ordered idx offsets (in 16-wrapped columns, x8 replicas)
        pinfo = []
        pos = 0
        for pr_ in wpairs:
            wlo = sum(c_lo[w] for w in pr_)
            whi = sum(c_hi[w] for w in pr_)
            pinfo.append((pos, wlo, pos + 8 * wlo, whi))
            pos += 8 * (wlo + whi)
        return dict(call_w=call_w, off=off, total_call=total_call,
                    pinfo=pinfo, total8=pos,
                    maxlo=max(i[1] for i in pinfo),
                    maxhi=max(i[3] for i in pinfo),
                    maxcall=max(call_w))

    geom_wr = rel_geom(c_lo_wr, c_hi_wr)
    geom_wn = rel_geom(c_lo_wn, c_hi_wn)

    nc = bacc.Bacc("TRN2", target_bir_lowering=False, debug=False,
                   num_swdge_queues=nq)

    TW = P if use_fp8 else D
    xa8 = nc.dram_tensor("xa8", [N, TW], bf16, kind="ExternalInput")
    xp8 = nc.dram_tensor("xp8", [N, TW], bf16, kind="ExternalInput")
    xta = nc.dram_tensor("xta", [D, npad], bf16, kind="ExternalInput")
    xtp = nc.dram_tensor("xtp", [D, npad], bf16, kind="ExternalInput")

    wnames = []
    for tag in ("wr", "wn"):
        wnames += [f"wp_self_{tag}", f"wp_rel_{tag}", f"wq_rel_{tag}",
                   f"wf_self_{tag}", f"wf_rel_{tag}"]
    wdram = {n: nc.dram_tensor(n, [D, D], bf16, kind="ExternalInput") for n in wnames}
    brow_d = {tag: nc.dram_tensor(f"brows_{tag}", [1, 3 * D], bf16,
                                  kind="ExternalInput") for tag in ("wr", "wn")}
    wrep_d = nc.dram_tensor("wrep", [P, D], f32, kind="ExternalInput")
    iota_d = nc.dram_tensor("iota", [P, P], bf16, kind="ExternalInput")
    ident_d = nc.dram_tensor("ident", [P, P], bf16, kind="ExternalInput")
    ones_d = nc.dram_tensor("ones", [1, P], bf16, kind="ExternalInput")

    idx_wr_d = nc.dram_tensor("idx_wr", [P, geom_wr["total8"]], i16, kind="ExternalInput")
    idx_wn_d = nc.dram_tensor("idx_wn", [P, geom_wn["total8"]], i16, kind="ExternalInput")
    colf_wr_d = nc.dram_tensor("colf_wr", [P, geom_wr["total_call"]], bf16, kind="ExternalInput")
    colf_wn_d = nc.dram_tensor("colf_wn", [P, geom_wn["total_call"]], bf16, kind="ExternalInput")
    recip_wr_d = nc.dram_tensor("recip_wr", [P, nwin], f32, kind="ExternalInput")
    recip_wn_d = nc.dram_tensor("recip_wn", [P, nwin], f32, kind="ExternalInput")

    oa = nc.dram_tensor("oa", [npad, D], bf16, kind="ExternalOutput")
    op_ = nc.dram_tensor("op", [npad, D], bf16, kind="ExternalOutput")

    with tile.TileContext(nc) as tc:
        with tc.tile_pool(name="const", bufs=1) as cpool, \
             tc.tile_pool(name="gbuf", bufs=gbufs) as gpool, \
             tc.tile_pool(name="oh", bufs=3) as ohpool, \
             tc.tile_pool(name="sb", bufs=3) as sbpool, \
             tc.tile_pool(name="mps", bufs=2, space="PSUM") as mpool, \
             tc.tile_pool(name="tps", bufs=1, space="PSUM") as tpool, \
             tc.tile_pool(name="dps", bufs=1, space="PSUM") as dpool:

            def load(dram, shape, dtype, tag):
                t = cpool.tile(shape, dtype, tag=tag)
                nc.sync.dma_start(t[:], dram)
                return t

            def load_consts():
                """(Re)load all constants; per scale-iteration so the
                scale-unrolled timing program repeats the full pipeline."""
                iota_t = load(iota_d[:], [P, P], bf16, "c_iota")
                ident_t = load(ident_d[:], [P, P], bf16, "c_ident")
                ones_t = load(ones_d[:], [1, P], bf16, "c_ones")
                wrep_t = load(wrep_d[:], [P, D], f32, "c_wrep")
                wt = {n: (load(wdram[n][0:P, :], [P, D], bf16, f"c_{n}0"),
                          load(wdram[n][P:D, :], [P, D], bf16, f"c_{n}1"))
                      for n in wnames}
                brow = {tag: load(brow_d[tag][:], [1, 3 * D], bf16, f"c_br{tag}")
                        for tag in ("wr", "wn")}
                xta_t = (load(xta[0:P, :], [P, npad], bf16, "c_xta0"),
                         load(xta[P:D, :], [P, npad], bf16, "c_xta1"))
                xtp_t = (load(xtp[0:P, :], [P, npad], bf16, "c_xtp0"),
                         load(xtp[P:D, :], [P, npad], bf16, "c_xtp1"))
                idx_wr_t = load(idx_wr_d[:], [P, geom_wr["total8"]], i16, "c_idxwr")
                idx_wn_t = load(idx_wn_d[:], [P, geom_wn["total8"]], i16, "c_idxwn")
                colf_wr_t = load(colf_wr_d[:], [P, geom_wr["total_call"]], bf16, "c_colfwr")
                colf_wn_t = load(colf_wn_d[:], [P, geom_wn["total_call"]], bf16, "c_colfwn")
                recip_wr_t = load(recip_wr_d[:], [P, nwin], f32, "c_recipwr")
                recip_wn_t = load(recip_wn_d[:], [P, nwin], f32, "c_recipwn")

                rels = [
                    dict(tag="wr", table=xa8, idx=idx_wr_t, colf=colf_wr_t,
                         recip=recip_wr_t, c_lo=c_lo_wr, c_hi=c_hi_wr,
                         geom=geom_wr, xt=xtp_t, q0=0, out=op_),
                    dict(tag="wn", table=xp8, idx=idx_wn_t, colf=colf_wn_t,
                         recip=recip_wn_t, c_lo=c_lo_wn, c_hi=c_hi_wn,
                         geom=geom_wn, xt=xta_t, q0=2, out=oa),
                ]
                for r in rels:
                    tag = r["tag"]
                    r["wp_self"] = wt[f"wp_self_{tag}"]
                    r["wp_rel"] = wt[f"wp_rel_{tag}"]
                    r["wq_rel"] = wt[f"wq_rel_{tag}"]
                    r["wf_self"] = wt[f"wf_self_{tag}"]
                    r["wf_rel"] = wt[f"wf_rel_{tag}"]
                    r["brow"] = brow[tag]
                return iota_t, ident_t, ones_t, wrep_t, rels

            def emit_pair_gather(ip, r):
                geom = r["geom"]
                io_lo, wlo, io_hi, whi = geom["pinfo"][ip]
                tag = r["tag"]
                g_lo = gpool.tile([P, geom["maxlo"], D], f8, tag=f"glo{tag}")
                g_hi = gpool.tile([P, geom["maxhi"], D], f8, tag=f"ghi{tag}")
                if not skip_gather:
                    n_lo, n_hi = wlo * P, whi * P
                    sl_lo, sl_hi = wlo, whi
                    if tiny_idx:
                        n_lo = n_hi = P
                        sl_lo = sl_hi = 1
                    out_lo = (g_lo.bitcast(bf16) if use_fp8 else g_lo)[:, 0:sl_lo, :]
                    nc.gpsimd.dma_gather(
                        out_lo, r["table"][:],
                        r["idx"][:, io_lo: io_lo + 8 * sl_lo],
                        n_lo, n_lo, TW, single_packet=single_packet,
                        queue_num=r["q0"] % nq)
                    out_hi = (g_hi.bitcast(bf16) if use_fp8 else g_hi)[:, 0:sl_hi, :]
                    nc.gpsimd.dma_gather(
                        out_hi, r["table"][HALF:, :],
                        r["idx"][:, io_hi: io_hi + 8 * sl_hi],
                        n_hi, n_hi, TW, single_packet=single_packet,
                        queue_num=(r["q0"] + 1) % nq)
                return g_lo, g_hi

            def emit_window(w, r, g_lo, g_hi, lo0, hi0):
                tag = r["tag"]
                geom = r["geom"]
                c_lo, c_hi = r["c_lo"][w], r["c_hi"][w]
                call = c_lo + c_hi
                co = geom["off"][w]

                oh = ohpool.tile([P, geom["maxcall"], P], f8, tag=f"oh{tag}")
                nc.vector.tensor_tensor(
                    out=oh[:, 0:call, :],
                    in0=r["colf"][:, co: co + call, None].to_broadcast([P, call, P]),
                    in1=iota_t[:, None, :].to_broadcast([P, call, P]),
                    op=OP.is_equal)

                m_ps = mpool.tile([P, D], f32, tag="m")
                for k in range(call):
                    rhs = (g_lo[:, lo0 + k, :] if k < c_lo
                           else g_hi[:, hi0 + k - c_lo, :])
                    nc.tensor.matmul(out=m_ps[:], lhsT=oh[:, k, :], rhs=rhs,
                                     start=(k == 0), stop=(k == call - 1))

                # deg-normalize on the scalar engine (per-dst 1/deg scale)
                m_sb = sbpool.tile([P, D], bf16, tag="m_sb")
                nc.scalar.activation(out=m_sb[:], in_=m_ps[:], func=AF.Copy,
                                     scale=r["recip"][:, w: w + 1])

                mt = []
                for h2 in range(2):
                    t_ps = tpool.tile([P, P], bf16, tag=f"t{h2}")
                    nc.tensor.transpose(out=t_ps[:],
                                        in_=m_sb[:, h2 * P: (h2 + 1) * P],
                                        identity=ident_t[:])
                    mt_sb = sbpool.tile([P, P], bf16, tag=f"mt{h2}")
                    nc.scalar.activation(out=mt_sb[:], in_=t_ps[:], func=AF.Copy)
                    mt.append(mt_sb)

                xsl0 = r["xt"][0][:, w * P: (w + 1) * P]
                xsl1 = r["xt"][1][:, w * P: (w + 1) * P]
                br = r["brow"]

                def dense(ps, parts, brow_slice):
                    for i, (lhsT, rhs) in enumerate(parts):
                        nc.tensor.matmul(out=ps, lhsT=lhsT, rhs=rhs,
                                         start=(i == 0), stop=False)
                    nc.tensor.matmul(out=ps, lhsT=ones_t[:], rhs=brow_slice,
                                     start=False, stop=True)
                    return ps

                pt = dpool.tile([P, D], f32, tag="p")
                qt = dpool.tile([P, D], f32, tag="q")
                zht = dpool.tile([P, D], f32, tag="zh")
                zat = dpool.tile([P, D], f32, tag="za")
                p_ps = dense(pt[:],
                             [(xsl0, r["wp_self"][0][:]), (xsl1, r["wp_self"][1][:]),
                              (mt[0][:], r["wp_rel"][0][:]), (mt[1][:], r["wp_rel"][1][:])],
                             br[:, 0:D])
                q_ps = dense(qt[:],
                             [(xsl0, r["wp_self"][0][:]), (xsl1, r["wp_self"][1][:]),
                              (mt[0][:], r["wq_rel"][0][:]), (mt[1][:], r["wq_rel"][1][:])],
                             br[:, 0:D])
                zh_ps = dense(zht[:],
                              [(xsl0, r["wf_self"][0][:]), (xsl1, r["wf_self"][1][:])],
                              br[:, D:2 * D])
                za_ps = dense(zat[:],
                              [(mt[0][:], r["wf_rel"][0][:]), (mt[1][:], r["wf_rel"][1][:])],
                              br[:, 2 * D:3 * D])

                th = sbpool.tile([P, D], f32, tag="th")
                nc.scalar.activation(out=th[:], in_=zh_ps, func=AF.Tanh)
                ta = sbpool.tile([P, D], f32, tag="ta")
                nc.scalar.activation(out=ta[:], in_=za_ps, func=AF.Tanh)

                v = sbpool.tile([P, D], f32, tag="v")
                nc.vector.tensor_tensor(out=v[:], in0=th[:], in1=ta[:],
                                        op=OP.subtract)
                vw = sbpool.tile([P, D], f32, tag="vw")
                nc.vector.tensor_tensor(out=vw[:], in0=v[:], in1=wrep_t[:],
                                        op=OP.mult)
                dsc = sbpool.tile([P, 1], f32, tag="dsc")
                nc.vector.tensor_reduce(out=dsc[:], in_=vw[:],
                                        axis=mybir.AxisListType.X, op=OP.add)

                t_sc = sbpool.tile([P, 1], f32, tag="tsc")
                nc.scalar.activation(out=t_sc[:], in_=dsc[:], func=AF.Tanh)

                wq = sbpool.tile([P, D], f32, tag="wq")
                nc.vector.tensor_scalar(out=wq[:], in0=q_ps,
                                        scalar1=t_sc[:, 0:1], scalar2=None,
                                        op0=OP.mult)
                outt = sbpool.tile([P, D], bf16, tag="outt")
                nc.vector.tensor_tensor(out=outt[:], in0=wq[:], in1=p_ps,
                                        op=OP.add)
                nc.sync.dma_start(r["out"][w * P: (w + 1) * P, :], outt[:])

            last_g = None
            for _s in range(scale):
                iota_t, ident_t, ones_t, wrep_t, rels = load_consts()
                for ip, pr_ in enumerate(wpairs):
                    gt = {r["tag"]: emit_pair_gather(ip, r) for r in rels}
                    last_g = gt[rels[0]["tag"]][0]
                    if gather_only:
                        continue
                    for j, w in enumerate(pr_):
                        for r in rels:
                            g_lo, g_hi = gt[r["tag"]]
                            lo0 = sum(r["c_lo"][v] for v in pr_[:j])
                            hi0 = sum(r["c_hi"][v] for v in pr_[:j])
                            emit_window(w, r, g_lo, g_hi, lo0, hi0)
            if gather_only:
                fin = sbpool.tile([P, D], bf16, tag="fin")
                nc.vector.tensor_copy(out=fin[:], in_=last_g[:, 0, :])
                nc.sync.dma_start(oa[0:P, :], fin[:])

    nc.compile()
    return nc


# ---------------------------------------------------------------- driver
_PROG_CACHE = {}


def _get_program(key):
    if key not in _PROG_CACHE:
        _PROG_CACHE[key] = build_program(*key)
    return _PROG_CACHE[key]


def _make_in_maps(pr):
    shared = dict(
        xa8=pr["xa8"], xp8=pr["xp8"],
        iota=pr["iota"], ident=pr["ident"], ones=pr["ones"], wrep=pr["wrep"],
        brows_wr=pr["brows_wr"], brows_wn=pr["brows_wn"],
    )
    for tag in ("wr", "wn"):
        for nm in ("wp_self", "wp_rel", "wq_rel", "wf_self", "wf_rel"):
            shared[f"{nm}_{tag}"] = pr[f"{nm}_{tag}"]
    idx_wr, colf_wr, recip_wr, _, _ = pr["wr"]
    idx_wn, colf_wn, recip_wn, _, _ = pr["wn"]
    in_maps = []
    for c in range(NCORES):
        w0, w1 = c * NWIN, (c + 1) * NWIN
        m = dict(shared)
        m["xta"] = pr["xta"][c]
        m["xtp"] = pr["xtp"][c]
        m["idx_wr"] = np.ascontiguousarray(np.tile(idx_wr[c], (8, 1)))
        m["idx_wn"] = np.ascontiguousarray(np.tile(idx_wn[c], (8, 1)))
        m["colf_wr"] = np.ascontiguousarray(colf_wr[c]).astype(BF16)
        m["colf_wn"] = np.ascontiguousarray(colf_wn[c]).astype(BF16)
        m["recip_wr"] = np.ascontiguousarray(recip_wr[:, w0:w1])
        m["recip_wn"] = np.ascontiguousarray(recip_wn[:, w0:w1])
        in_maps.append(m)
    return in_maps


def run(trace=False, tmpdir=None, **inputs):
    pr = _host_prep(inputs)
    _, _, _, c_lo_wr, c_hi_wr = pr["wr"]
    _, _, _, c_lo_wn, c_hi_wn = pr["wn"]
    nc = _get_program((NWIN, c_lo_wr, c_hi_wr, c_lo_wn, c_hi_wn, 1, NQ, USE_FP8))
    in_maps = _make_in_maps(pr)
    res = run_bass_kernel_spmd(nc, in_maps, list(range(NCORES)),
                               trace=trace, tmpdir=tmpdir)
    oa = np.empty((N, D), dtype=F32)
    op = np.empty((N, D), dtype=F32)
    for c in range(NCORES):
        r0, r1 = c * NPAD, min(N, (c + 1) * NPAD)
        oa[r0:r1] = res.results[c]["oa"][: r1 - r0].astype(F32)
        op[r0:r1] = res.results[c]["op"][: r1 - r0].astype(F32)
    return (oa, op), res


def kernel(**inputs):
    (oa, op), _ = run(trace=False, **inputs)
    return (oa, op)



# revision 18
# speedup vs baseline: 6.9666x; 1.1647x over previous
"""HANConv Trainium2 kernel (8 NeuronCores, SPMD, full-I/O contract).

Strategy (v2)
-------------
Destination-sharded, fully core-independent:
  * Each core owns 1/8 of destination nodes for BOTH relations
    (writes: author->paper, written: paper->author).
  * Edges are sorted by (dst window, src half, src) on host. Per window,
    source rows are gathered as fp8(e4m3) 256B rows via gpsimd.dma_gather,
    round-robin over 4 SWDGE queues (4x the single-queue descriptor
    throughput; the gather is descriptor-bound, so fp8 also halves bytes),
    and segment-summed with fp8 one-hot matmuls accumulating in f32 PSUM.
  * Aggregating RAW features (M = A @ x, then per-dst 1/deg scale on the
    scalar engine) lets every later transform be a dense matmul from M with
    host-folded weights, so no cross-core exchange is ever needed.
  * 2-candidate semantic softmax is rewritten tanh-only:
        out = p + tanh(0.5*(s_h - s_agg)) * q
        p = 0.5*(h + agg),  q = 0.5*(h - agg)
    with the 0.5 factors folded into the weights on host. The scalar
    engine therefore never switches activation tables.
  * Scores use one fused DVE tensor_tensor_reduce:
        dsc = 0.5 * sum(w_score * (tanh(z_h) - tanh(z_agg)))
  * Self path computed from host-transposed x slices with folded weights.
  * Outputs written bf16 and upcast to f32 on host.
"""

import sys

sys.path.insert(0, "/opt/trn_rl_repo")

import numpy as np
import ml_dtypes

import concourse.bacc as bacc
import concourse.mybir as mybir
import concourse.tile as tile
from concourse.bass_utils import run_bass_kernel_spmd

P = 128
N = 50000
D = 256
HALF = 32768  # int16 gather index limit
NCORES = 8
NW_TOTAL = (N + P - 1) // P            # 391 destination windows
NWIN = (NW_TOTAL + NCORES - 1) // NCORES  # 49 windows per core
NW_ALLOC = NWIN * NCORES               # 392 (incl. 1 phantom window)
NPAD = NWIN * P                        # 6272 output rows per core

BF16 = ml_dtypes.bfloat16
FP8 = ml_dtypes.float8_e4m3
F32 = np.float32

USE_FP8 = True
NQ = 4


GROUP = 2


def _pairs(group=None):
    """Window slots grouped into gather groups of `group` windows."""
    g = GROUP if group is None else group
    out = []
    w = 0
    while w < NWIN:
        out.append(tuple(range(w, min(w + g, NWIN))))
        w += g
    return out


# ---------------------------------------------------------------- host prep
def _prep_relation(row, col):
    """Sort edges by (dst window, src half, src); per-slot dynamic widths.

    Slot widths c_lo/c_hi[w] are the max over the 8 cores so the SPMD
    program is common. Gathers are issued per window PAIR (lo and hi
    halves separately) so the idx layout per core is, in pair order:
      [pair lo: slots w0|w1 ...][pair hi: slots w0|w1 ...] ...
    colf layout per core is per-slot: [slot: lo blocks | hi blocks] ...

    Returns (idx16_percore [NCORES,16,total8], colf_percore
    [NCORES,P,total_call], recip [P,NW_ALLOC], c_lo[NWIN], c_hi[NWIN]).
    """
    E = row.shape[0]
    key = (col // P) * 2 + (row >= HALF)
    order = np.lexsort((row, key))
    ks = key[order]
    rs = row[order].astype(np.int64)
    cs = col[order].astype(np.int64)

    counts = np.bincount(key, minlength=NW_ALLOC * 2).astype(np.int64)
    lo_cnt = counts[0::2].reshape(NCORES, NWIN)
    hi_cnt = counts[1::2].reshape(NCORES, NWIN)
    c_lo = np.maximum(1, -(-lo_cnt.max(axis=0) // P))  # [NWIN]
    c_hi = np.maximum(1, -(-hi_cnt.max(axis=0) // P))  # [NWIN]
    call_w = c_lo + c_hi
    off = np.zeros(NWIN + 1, dtype=np.int64)
    off[1:] = np.cumsum(call_w)
    total_call = int(off[-1])

    # idx layout offsets (in index units) per (slot, half), pair-ordered
    idx_base = np.zeros((NWIN, 2), dtype=np.int64)
    pos = 0
    for pr_ in _pairs():
        for w in pr_:
            idx_base[w, 0] = pos
            pos += int(c_lo[w]) * P
        for w in pr_:
            idx_base[w, 1] = pos
            pos += int(c_hi[w]) * P
    total_idx = pos

    grp_start = np.zeros(NW_ALLOC * 2 + 1, dtype=np.int64)
    np.cumsum(counts, out=grp_start[1:])
    rank = np.arange(E, dtype=np.int64) - grp_start[ks]
    w_of = ks // 2
    core = w_of // NWIN
    slot = w_of % NWIN
    hi_of = ks % 2

    idx_flat = np.zeros(NCORES * total_idx, dtype=np.int16)
    ipos = core * total_idx + idx_base[slot, hi_of] + rank
    idx_flat[ipos] = (rs - HALF * hi_of).astype(np.int16)
    col_flat = np.full(NCORES * total_call * P, -1.0, dtype=F32)
    cpos = core * (total_call * P) + (off[slot] + hi_of * c_lo[slot]) * P + rank
    col_flat[cpos] = (cs - w_of * P).astype(F32)

    # wrap idx per gather region: region r of length L -> [16, L*8/16...]
    idx_pc = idx_flat.reshape(NCORES, total_idx)
    parts = []
    pos = 0
    for pr_ in _pairs():
        for half, carr in ((0, c_lo), (1, c_hi)):
            L = int(sum(carr[w] for w in pr_)) * P
            reg = idx_pc[:, pos: pos + L]
            parts.append(reg.reshape(NCORES, L // 16, 16).transpose(0, 2, 1))
            pos += L
    idx16 = np.concatenate(parts, axis=2)  # [NCORES, 16, total_idx//16]

    colf = col_flat.reshape(NCORES, total_call, P).transpose(0, 2, 1)

    deg = np.bincount(col, minlength=NW_ALLOC * P).astype(F32)[: NW_ALLOC * P]
    recip = (1.0 / np.maximum(deg, 1.0)).reshape(NW_ALLOC, P).T  # [P, NW]
    return idx16, colf, recip, tuple(int(x) for x in c_lo), tuple(int(x) for x in c_hi)


def _host_prep(inp):
    pr = {}
    pr["wr"] = _prep_relation(np.asarray(inp["row_writes"]), np.asarray(inp["col_writes"]))
    pr["wn"] = _prep_relation(np.asarray(inp["row_written"]), np.asarray(inp["col_written"]))

    xa = np.asarray(inp["x_author"], dtype=F32)
    xp = np.asarray(inp["x_paper"], dtype=F32)
    if USE_FP8:
        # fp8 gather tables, viewed as bf16 [N, 128] for the byte-moving gather
        pr["xa8"] = xa.astype(FP8).view(np.uint16).view(BF16)
        pr["xp8"] = xp.astype(FP8).view(np.uint16).view(BF16)
    else:
        pr["xa8"] = xa.astype(BF16)
        pr["xp8"] = xp.astype(BF16)

    # per-core transposed x slices (for the self path of the dst shard)
    xta, xtp = [], []
    for c in range(NCORES):
        r0, r1 = c * NPAD, min(N, (c + 1) * NPAD)
        sa = np.zeros((D, NPAD), dtype=BF16)
        sp = np.zeros((D, NPAD), dtype=BF16)
        sa[:, : r1 - r0] = xa[r0:r1].T
        sp[:, : r1 - r0] = xp[r0:r1].T
        xta.append(sa)
        xtp.append(sp)
    pr["xta"], pr["xtp"] = xta, xtp

    W_sem = np.asarray(inp["W_sem"], dtype=F32)
    b_sem = np.asarray(inp["b_sem"], dtype=F32)
    w_score = np.asarray(inp["w_score"], dtype=F32)

    def w(name):
        return np.asarray(inp[name], dtype=F32)

    # folded weights per relation: (dst self weight, rel weight)
    for tag, wself, bself, wrel in (
        ("wr", w("W_self_paper"), w("b_self_paper"), w("W_rel_writes")),
        ("wn", w("W_self_author"), w("b_self_author"), w("W_rel_written")),
    ):
        pr[f"wp_self_{tag}"] = (0.5 * wself).astype(BF16)
        pr[f"wp_rel_{tag}"] = (0.5 * wrel).astype(BF16)
        pr[f"wq_rel_{tag}"] = (-0.5 * wrel).astype(BF16)
        pr[f"wf_self_{tag}"] = (wself @ W_sem).astype(BF16)
        pr[f"wf_rel_{tag}"] = (wrel @ W_sem).astype(BF16)
        # bias rows: [1, 3*D] = (0.5*b_self | b_self@W_sem + b_sem | b_sem)
        pr[f"brows_{tag}"] = np.concatenate([
            0.5 * bself, bself @ W_sem + b_sem, b_sem,
        ]).reshape(1, 3 * D).astype(BF16)

    # pre-scaled by the 0.5 from sigmoid(x) = 0.5*(1+tanh(x/2))
    pr["wrep"] = np.tile(0.5 * w_score, (P, 1)).astype(F32)
    pr["iota"] = np.tile(np.arange(P, dtype=F32), (P, 1)).astype(BF16)
    pr["ident"] = np.eye(P, dtype=F32).astype(BF16)
    pr["ones"] = np.ones((1, P), dtype=BF16)
    return pr


# ---------------------------------------------------------------- program
def build_program(nwin, c_lo_wr, c_hi_wr, c_lo_wn, c_hi_wn, scale=1,
                  nq=4, use_fp8=True, skip_gather=False, gather_only=False,
                  group=None, single_packet=False, tiny_idx=False,
                  gbufs=3, reload_consts=True, stream_consts=False):
    f32 = mybir.dt.float32
    bf16 = mybir.dt.bfloat16
    f8 = mybir.dt.float8e4 if use_fp8 else mybir.dt.bfloat16
    i16 = mybir.dt.int16
    AF = mybir.ActivationFunctionType
    OP = mybir.AluOpType

    npad = nwin * P

    g_ = GROUP if group is None else group

    def pairs_of(n):
        out = []
        w = 0
        while w < n:
            out.append(tuple(range(w, min(w + g_, n))))
            w += g_
        return out

    wpairs = pairs_of(nwin)

    def rel_geom(c_lo, c_hi):
        call_w = [c_lo[w] + c_hi[w] for w in range(nwin)]
        off = [0]
        for w in range(nwin):
            off.append(off[-1] + call_w[w])
        total_call = off[-1]
        # pair-ordered idx offsets (in 16-wrapped columns, x8 replicas)
        pinfo = []
        pos = 0
        for pr_ in wpairs:
            wlo = sum(c_lo[w] for w in pr_)
            whi = sum(c_hi[w] for w in pr_)
            pinfo.append((pos, wlo, pos + 8 * wlo, whi))
            pos += 8 * (wlo + whi)
        return dict(call_w=call_w, off=off, total_call=total_call,
                    pinfo=pinfo, total8=pos,
                    maxlo=max(i[1] for i in pinfo),
                    maxhi=max(i[3] for i in pinfo),
                    maxcall=max(call_w))

    geom_wr = rel_geom(c_lo_wr, c_hi_wr)
    geom_wn = rel_geom(c_lo_wn, c_hi_wn)

    nc = bacc.Bacc("TRN2", target_bir_lowering=False, debug=False,
                   num_swdge_queues=nq)

    TW = P if use_fp8 else D
    xa8 = nc.dram_tensor("xa8", [N, TW], bf16, kind="ExternalInput")
    xp8 = nc.dram_tensor("xp8", [N, TW], bf16, kind="ExternalInput")
    xta = nc.dram_tensor("xta", [D, npad], bf16, kind="ExternalInput")
    xtp = nc.dram_tensor("xtp", [D, npad], bf16, kind="ExternalInput")

    wnames = []
    for tag in ("wr", "wn"):
        wnames += [f"wp_self_{tag}", f"wp_rel_{tag}", f"wq_rel_{tag}",
                   f"wf_self_{tag}", f"wf_rel_{tag}"]
    wdram = {n: nc.dram_tensor(n, [D, D], bf16, kind="ExternalInput") for n in wnames}
    brow_d = {tag: nc.dram_tensor(f"brows_{tag}", [1, 3 * D], bf16,
                                  kind="ExternalInput") for tag in ("wr", "wn")}
    wrep_d = nc.dram_tensor("wrep", [P, D], f32, kind="ExternalInput")
    iota_d = nc.dram_tensor("iota", [P, P], bf16, kind="ExternalInput")
    ident_d = nc.dram_tensor("ident", [P, P], bf16, kind="ExternalInput")
    ones_d = nc.dram_tensor("ones", [1, P], bf16, kind="ExternalInput")

    idx_wr_d = nc.dram_tensor("idx_wr", [P, geom_wr["total8"]], i16, kind="ExternalInput")
    idx_wn_d = nc.dram_tensor("idx_wn", [P, geom_wn["total8"]], i16, kind="ExternalInput")
    colf_wr_d = nc.dram_tensor("colf_wr", [P, geom_wr["total_call"]], bf16, kind="ExternalInput")
    colf_wn_d = nc.dram_tensor("colf_wn", [P, geom_wn["total_call"]], bf16, kind="ExternalInput")
    recip_wr_d = nc.dram_tensor("recip_wr", [P, nwin], f32, kind="ExternalInput")
    recip_wn_d = nc.dram_tensor("recip_wn", [P, nwin], f32, kind="ExternalInput")

    oa = nc.dram_tensor("oa", [npad, D], bf16, kind="ExternalOutput")
    op_ = nc.dram_tensor("op", [npad, D], bf16, kind="ExternalOutput")

    with tile.TileContext(nc) as tc:
        with tc.tile_pool(name="const", bufs=1) as cpool, \
             tc.tile_pool(name="strm", bufs=2) as stpool, \
             tc.tile_pool(name="gbuf", bufs=gbufs) as gpool, \
             tc.tile_pool(name="oh", bufs=3) as ohpool, \
             tc.tile_pool(name="sb", bufs=3) as sbpool, \
             tc.tile_pool(name="mps", bufs=2, space="PSUM") as mpool, \
             tc.tile_pool(name="tps", bufs=1, space="PSUM") as tpool, \
             tc.tile_pool(name="dps", bufs=1, space="PSUM") as dpool:

            def load(dram, shape, dtype, tag, pool=None):
                if pool is stpool and not stream_consts:
                    pool = cpool
                t = (pool or cpool).tile(shape, dtype, tag=tag)
                nc.sync.dma_start(t[:], dram)
                return t

            def load_consts():
                """(Re)load all constants; per scale-iteration so the
                scale-unrolled timing program repeats the full pipeline."""
                iota_t = load(iota_d[:], [P, P], bf16, "c_iota")
                ident_t = load(ident_d[:], [P, P], bf16, "c_ident")
                ones_t = load(ones_d[:], [1, P], bf16, "c_ones")
                wrep_t = load(wrep_d[:], [P, D], f32, "c_wrep")
                wt = {n: (load(wdram[n][0:P, :], [P, D], bf16, f"c_{n}0"),
                          load(wdram[n][P:D, :], [P, D], bf16, f"c_{n}1"))
                      for n in wnames}
                brow = {tag: load(brow_d[tag][:], [1, 3 * D], bf16, f"c_br{tag}")
                        for tag in ("wr", "wn")}
                xta_t = (load(xta[0:P, :], [P, npad], bf16, "c_xta0"),
                         load(xta[P:D, :], [P, npad], bf16, "c_xta1"))
                xtp_t = (load(xtp[0:P, :], [P, npad], bf16, "c_xtp0"),
                         load(xtp[P:D, :], [P, npad], bf16, "c_xtp1"))
                idx_wr_t = load(idx_wr_d[:], [P, geom_wr["total8"]], i16, "c_idxwr",
                                pool=stpool)
                idx_wn_t = load(idx_wn_d[:], [P, geom_wn["total8"]], i16, "c_idxwn",
                                pool=stpool)
                colf_wr_t = load(colf_wr_d[:], [P, geom_wr["total_call"]], bf16,
                                 "c_colfwr", pool=stpool)
                colf_wn_t = load(colf_wn_d[:], [P, geom_wn["total_call"]], bf16,
                                 "c_colfwn", pool=stpool)
                recip_wr_t = load(recip_wr_d[:], [P, nwin], f32, "c_recipwr",
                                  pool=stpool)
                recip_wn_t = load(recip_wn_d[:], [P, nwin], f32, "c_recipwn",
                                  pool=stpool)

                rels = [
                    dict(tag="wr", table=xa8, idx=idx_wr_t, colf=colf_wr_t,
                         recip=recip_wr_t, c_lo=c_lo_wr, c_hi=c_hi_wr,
                         geom=geom_wr, xt=xtp_t, q0=0, out=op_),
                    dict(tag="wn", table=xp8, idx=idx_wn_t, colf=colf_wn_t,
                         recip=recip_wn_t, c_lo=c_lo_wn, c_hi=c_hi_wn,
                         geom=geom_wn, xt=xta_t, q0=2, out=oa),
                ]
                for r in rels:
                    tag = r["tag"]
                    r["wp_self"] = wt[f"wp_self_{tag}"]
                    r["wp_rel"] = wt[f"wp_rel_{tag}"]
                    r["wq_rel"] = wt[f"wq_rel_{tag}"]
                    r["wf_self"] = wt[f"wf_self_{tag}"]
                    r["wf_rel"] = wt[f"wf_rel_{tag}"]
                    r["brow"] = brow[tag]
                return iota_t, ident_t, ones_t, wrep_t, rels

            def emit_pair_gather(ip, r):
                geom = r["geom"]
                io_lo, wlo, io_hi, whi = geom["pinfo"][ip]
                tag = r["tag"]
                g_lo = gpool.tile([P, geom["maxlo"], D], f8, tag=f"glo{tag}")
                g_hi = gpool.tile([P, geom["maxhi"], D], f8, tag=f"ghi{tag}")
                if not skip_gather:
                    n_lo, n_hi = wlo * P, whi * P
                    sl_lo, sl_hi = wlo, whi
                    if tiny_idx:
                        n_lo = n_hi = P
                        sl_lo = sl_hi = 1
                    out_lo = (g_lo.bitcast(bf16) if use_fp8 else g_lo)[:, 0:sl_lo, :]
                    nc.gpsimd.dma_gather(
                        out_lo, r["table"][:],
                        r["idx"][:, io_lo: io_lo + 8 * sl_lo],
                        n_lo, n_lo, TW, single_packet=single_packet,
                        queue_num=r["q0"] % nq)
                    out_hi = (g_hi.bitcast(bf16) if use_fp8 else g_hi)[:, 0:sl_hi, :]
                    nc.gpsimd.dma_gather(
                        out_hi, r["table"][HALF:, :],
                        r["idx"][:, io_hi: io_hi + 8 * sl_hi],
                        n_hi, n_hi, TW, single_packet=single_packet,
                        queue_num=(r["q0"] + 1) % nq)
                return g_lo, g_hi

            def emit_window(w, r, g_lo, g_hi, lo0, hi0):
                tag = r["tag"]
                geom = r["geom"]
                c_lo, c_hi = r["c_lo"][w], r["c_hi"][w]
                call = c_lo + c_hi
                co = geom["off"][w]

                oh = ohpool.tile([P, geom["maxcall"], P], f8, tag=f"oh{tag}")
                nc.vector.tensor_tensor(
                    out=oh[:, 0:call, :],
                    in0=r["colf"][:, co: co + call, None].to_broadcast([P, call, P]),
                    in1=iota_t[:, None, :].to_broadcast([P, call, P]),
                    op=OP.is_equal)

                m_ps = mpool.tile([P, D], f32, tag="m")
                for k in range(call):
                    rhs = (g_lo[:, lo0 + k, :] if k < c_lo
                           else g_hi[:, hi0 + k - c_lo, :])
                    nc.tensor.matmul(out=m_ps[:], lhsT=oh[:, k, :], rhs=rhs,
                                     start=(k == 0), stop=(k == call - 1))

                # deg-normalize on the scalar engine (per-dst 1/deg scale)
                m_sb = sbpool.tile([P, D], bf16, tag="m_sb")
                nc.scalar.activation(out=m_sb[:], in_=m_ps[:], func=AF.Copy,
                                     scale=r["recip"][:, w: w + 1])

                mt = []
                for h2 in range(2):
                    t_ps = tpool.tile([P, P], bf16, tag=f"t{h2}")
                    nc.tensor.transpose(out=t_ps[:],
                                        in_=m_sb[:, h2 * P: (h2 + 1) * P],
                                        identity=ident_t[:])
                    mt_sb = sbpool.tile([P, P], bf16, tag=f"mt{h2}")
                    nc.scalar.activation(out=mt_sb[:], in_=t_ps[:], func=AF.Copy)
                    mt.append(mt_sb)

                xsl0 = r["xt"][0][:, w * P: (w + 1) * P]
                xsl1 = r["xt"][1][:, w * P: (w + 1) * P]
                br = r["brow"]

                def dense(ps, parts, brow_slice):
                    for i, (lhsT, rhs) in enumerate(parts):
                        nc.tensor.matmul(out=ps, lhsT=lhsT, rhs=rhs,
                                         start=(i == 0), stop=False)
                    nc.tensor.matmul(out=ps, lhsT=ones_t[:], rhs=brow_slice,
                                     start=False, stop=True)
                    return ps

                pt = dpool.tile([P, D], f32, tag="p")
                qt = dpool.tile([P, D], f32, tag="q")
                zht = dpool.tile([P, D], f32, tag="zh")
                zat = dpool.tile([P, D], f32, tag="za")
                p_ps = dense(pt[:],
                             [(xsl0, r["wp_self"][0][:]), (xsl1, r["wp_self"][1][:]),
                              (mt[0][:], r["wp_rel"][0][:]), (mt[1][:], r["wp_rel"][1][:])],
                             br[:, 0:D])
                q_ps = dense(qt[:],
                             [(xsl0, r["wp_self"][0][:]), (xsl1, r["wp_self"][1][:]),
                              (mt[0][:], r["wq_rel"][0][:]), (mt[1][:], r["wq_rel"][1][:])],
                             br[:, 0:D])
                zh_ps = dense(zht[:],
                              [(xsl0, r["wf_self"][0][:]), (xsl1, r["wf_self"][1][:])],
                              br[:, D:2 * D])
                za_ps = dense(zat[:],
                              [(mt[0][:], r["wf_rel"][0][:]), (mt[1][:], r["wf_rel"][1][:])],
                              br[:, 2 * D:3 * D])

                th = sbpool.tile([P, D], f32, tag="th")
                nc.scalar.activation(out=th[:], in_=zh_ps, func=AF.Tanh)
                ta = sbpool.tile([P, D], f32, tag="ta")
                nc.scalar.activation(out=ta[:], in_=za_ps, func=AF.Tanh)

                v = sbpool.tile([P, D], f32, tag="v")
                nc.vector.tensor_tensor(out=v[:], in0=th[:], in1=ta[:],
                                        op=OP.subtract)
                vw = sbpool.tile([P, D], f32, tag="vw")
                nc.vector.tensor_tensor(out=vw[:], in0=v[:], in1=wrep_t[:],
                                        op=OP.mult)
                dsc = sbpool.tile([P, 1], f32, tag="dsc")
                nc.vector.tensor_reduce(out=dsc[:], in_=vw[:],
                                        axis=mybir.AxisListType.X, op=OP.add)

                t_sc = sbpool.tile([P, 1], f32, tag="tsc")
                nc.scalar.activation(out=t_sc[:], in_=dsc[:], func=AF.Tanh)

                wq = sbpool.tile([P, D], f32, tag="wq")
                nc.vector.tensor_scalar(out=wq[:], in0=q_ps,
                                        scalar1=t_sc[:, 0:1], scalar2=None,
                                        op0=OP.mult)
                outt = sbpool.tile([P, D], bf16, tag="outt")
                nc.vector.tensor_tensor(out=outt[:], in0=wq[:], in1=p_ps,
                                        op=OP.add)
                nc.sync.dma_start(r["out"][w * P: (w + 1) * P, :], outt[:])

            last_g = None
            for _s in range(scale):
                if _s == 0 or reload_consts:
                    iota_t, ident_t, ones_t, wrep_t, rels = load_consts()
                for ip, pr_ in enumerate(wpairs):
                    gt = {r["tag"]: emit_pair_gather(ip, r) for r in rels}
                    last_g = gt[rels[0]["tag"]][0]
                    if gather_only:
                        continue
                    for j, w in enumerate(pr_):
                        for r in rels:
                            g_lo, g_hi = gt[r["tag"]]
                            lo0 = sum(r["c_lo"][v] for v in pr_[:j])
                            hi0 = sum(r["c_hi"][v] for v in pr_[:j])
                            emit_window(w, r, g_lo, g_hi, lo0, hi0)
            if gather_only:
                fin = sbpool.tile([P, D], bf16, tag="fin")
                nc.vector.tensor_copy(out=fin[:], in_=last_g[:, 0, :])
                nc.sync.dma_start(oa[0:P, :], fin[:])

    nc.compile()
    return nc


# ---------------------------------------------------------------- driver
_PROG_CACHE = {}


def _get_program(key):
    if key not in _PROG_CACHE:
        _PROG_CACHE[key] = build_program(*key)
    return _PROG_CACHE[key]


def _make_in_maps(pr):
    shared = dict(
        xa8=pr["xa8"], xp8=pr["xp8"],
        iota=pr["iota"], ident=pr["ident"], ones=pr["ones"], wrep=pr["wrep"],
        brows_wr=pr["brows_wr"], brows_wn=pr["brows_wn"],
    )
    for tag in ("wr", "wn"):
        for nm in ("wp_self", "wp_rel", "wq_rel", "wf_self", "wf_rel"):
            shared[f"{nm}_{tag}"] = pr[f"{nm}_{tag}"]
    idx_wr, colf_wr, recip_wr, _, _ = pr["wr"]
    idx_wn, colf_wn, recip_wn, _, _ = pr["wn"]
    in_maps = []
    for c in range(NCORES):
        w0, w1 = c * NWIN, (c + 1) * NWIN
        m = dict(shared)
        m["xta"] = pr["xta"][c]
        m["xtp"] = pr["xtp"][c]
        m["idx_wr"] = np.ascontiguousarray(np.tile(idx_wr[c], (8, 1)))
        m["idx_wn"] = np.ascontiguousarray(np.tile(idx_wn[c], (8, 1)))
        m["colf_wr"] = np.ascontiguousarray(colf_wr[c]).astype(BF16)
        m["colf_wn"] = np.ascontiguousarray(colf_wn[c]).astype(BF16)
        m["recip_wr"] = np.ascontiguousarray(recip_wr[:, w0:w1])
        m["recip_wn"] = np.ascontiguousarray(recip_wn[:, w0:w1])
        in_maps.append(m)
    return in_maps


def run(trace=False, tmpdir=None, **inputs):
    pr = _host_prep(inputs)
    _, _, _, c_lo_wr, c_hi_wr = pr["wr"]
    _, _, _, c_lo_wn, c_hi_wn = pr["wn"]
    nc = _get_program((NWIN, c_lo_wr, c_hi_wr, c_lo_wn, c_hi_wn, 1, NQ, USE_FP8))
    in_maps = _make_in_maps(pr)
    res = run_bass_kernel_spmd(nc, in_maps, list(range(NCORES)),
                               trace=trace, tmpdir=tmpdir)
    oa = np.empty((N, D), dtype=F32)
    op = np.empty((N, D), dtype=F32)
    for c in range(NCORES):
        r0, r1 = c * NPAD, min(N, (c + 1) * NPAD)
        oa[r0:r1] = res.results[c]["oa"][: r1 - r0].astype(F32)
        op[r0:r1] = res.results[c]["op"][: r1 - r0].astype(F32)
    return (oa, op), res


def kernel(**inputs):
    (oa, op), _ = run(trace=False, **inputs)
    return (oa, op)



# revision 19
# speedup vs baseline: 7.7038x; 1.1058x over previous
"""HANConv Trainium2 kernel (8 NeuronCores, SPMD, full-I/O contract).

Strategy (v2)
-------------
Destination-sharded, fully core-independent:
  * Each core owns 1/8 of destination nodes for BOTH relations
    (writes: author->paper, written: paper->author).
  * Edges are sorted by (dst window, src half, src) on host. Per window,
    source rows are gathered as fp8(e4m3) 256B rows via gpsimd.dma_gather,
    round-robin over 4 SWDGE queues (4x the single-queue descriptor
    throughput; the gather is descriptor-bound, so fp8 also halves bytes),
    and segment-summed with fp8 one-hot matmuls accumulating in f32 PSUM.
  * Aggregating RAW features (M = A @ x, then per-dst 1/deg scale on the
    scalar engine) lets every later transform be a dense matmul from M with
    host-folded weights, so no cross-core exchange is ever needed.
  * 2-candidate semantic softmax is rewritten tanh-only:
        out = p + tanh(0.5*(s_h - s_agg)) * q
        p = 0.5*(h + agg),  q = 0.5*(h - agg)
    with the 0.5 factors folded into the weights on host. The scalar
    engine therefore never switches activation tables.
  * Scores use one fused DVE tensor_tensor_reduce:
        dsc = 0.5 * sum(w_score * (tanh(z_h) - tanh(z_agg)))
  * Self path computed from host-transposed x slices with folded weights.
  * Outputs written bf16 and upcast to f32 on host.
"""

import sys

sys.path.insert(0, "/opt/trn_rl_repo")

import numpy as np
import ml_dtypes

import concourse.bacc as bacc
import concourse.mybir as mybir
import concourse.tile as tile
from concourse.bass_utils import run_bass_kernel_spmd

P = 128
N = 50000
D = 256
HALF = 32768  # int16 gather index limit
NCORES = 8
NW_TOTAL = (N + P - 1) // P            # 391 destination windows
NWIN = (NW_TOTAL + NCORES - 1) // NCORES  # 49 windows per core
NW_ALLOC = NWIN * NCORES               # 392 (incl. 1 phantom window)
NPAD = NWIN * P                        # 6272 output rows per core

BF16 = ml_dtypes.bfloat16
FP8 = ml_dtypes.float8_e4m3
F32 = np.float32

USE_FP8 = True
NQ = 4


GROUP = 2


def _pairs(group=None):
    """Window slots grouped into gather groups of `group` windows."""
    g = GROUP if group is None else group
    out = []
    w = 0
    while w < NWIN:
        out.append(tuple(range(w, min(w + g, NWIN))))
        w += g
    return out


# ---------------------------------------------------------------- host prep
def _prep_relation(row, col):
    """Sort edges by (dst window, src half, src); per-slot dynamic widths.

    Slot widths c_lo/c_hi[w] are the max over the 8 cores so the SPMD
    program is common. Gathers are issued per window PAIR (lo and hi
    halves separately) so the idx layout per core is, in pair order:
      [pair lo: slots w0|w1 ...][pair hi: slots w0|w1 ...] ...
    colf layout per core is per-slot: [slot: lo blocks | hi blocks] ...

    Returns (idx16_percore [NCORES,16,total8], colf_percore
    [NCORES,P,total_call], recip [P,NW_ALLOC], c_lo[NWIN], c_hi[NWIN]).
    """
    E = row.shape[0]
    key = (col // P) * 2 + (row >= HALF)
    order = np.lexsort((row, key))
    ks = key[order]
    rs = row[order].astype(np.int64)
    cs = col[order].astype(np.int64)

    counts = np.bincount(key, minlength=NW_ALLOC * 2).astype(np.int64)
    lo_cnt = counts[0::2].reshape(NCORES, NWIN)
    hi_cnt = counts[1::2].reshape(NCORES, NWIN)
    c_lo = np.maximum(1, -(-lo_cnt.max(axis=0) // P))  # [NWIN]
    c_hi = np.maximum(1, -(-hi_cnt.max(axis=0) // P))  # [NWIN]
    call_w = c_lo + c_hi
    off = np.zeros(NWIN + 1, dtype=np.int64)
    off[1:] = np.cumsum(call_w)
    total_call = int(off[-1])

    # idx layout offsets (in index units) per (slot, half), pair-ordered
    idx_base = np.zeros((NWIN, 2), dtype=np.int64)
    pos = 0
    for pr_ in _pairs():
        for w in pr_:
            idx_base[w, 0] = pos
            pos += int(c_lo[w]) * P
        for w in pr_:
            idx_base[w, 1] = pos
            pos += int(c_hi[w]) * P
    total_idx = pos

    grp_start = np.zeros(NW_ALLOC * 2 + 1, dtype=np.int64)
    np.cumsum(counts, out=grp_start[1:])
    rank = np.arange(E, dtype=np.int64) - grp_start[ks]
    w_of = ks // 2
    core = w_of // NWIN
    slot = w_of % NWIN
    hi_of = ks % 2

    idx_flat = np.zeros(NCORES * total_idx, dtype=np.int16)
    ipos = core * total_idx + idx_base[slot, hi_of] + rank
    idx_flat[ipos] = (rs - HALF * hi_of).astype(np.int16)
    col_flat = np.full(NCORES * total_call * P, -1.0, dtype=F32)
    cpos = core * (total_call * P) + (off[slot] + hi_of * c_lo[slot]) * P + rank
    col_flat[cpos] = (cs - w_of * P).astype(F32)

    # wrap idx per gather region: region r of length L -> [16, L*8/16...]
    idx_pc = idx_flat.reshape(NCORES, total_idx)
    parts = []
    pos = 0
    for pr_ in _pairs():
        for half, carr in ((0, c_lo), (1, c_hi)):
            L = int(sum(carr[w] for w in pr_)) * P
            reg = idx_pc[:, pos: pos + L]
            parts.append(reg.reshape(NCORES, L // 16, 16).transpose(0, 2, 1))
            pos += L
    idx16 = np.concatenate(parts, axis=2)  # [NCORES, 16, total_idx//16]

    colf = col_flat.reshape(NCORES, total_call, P).transpose(0, 2, 1)

    deg = np.bincount(col, minlength=NW_ALLOC * P).astype(F32)[: NW_ALLOC * P]
    recip = (1.0 / np.maximum(deg, 1.0)).reshape(NW_ALLOC, P).T  # [P, NW]
    return idx16, colf, recip, tuple(int(x) for x in c_lo), tuple(int(x) for x in c_hi)


def _host_prep(inp):
    pr = {}
    pr["wr"] = _prep_relation(np.asarray(inp["row_writes"]), np.asarray(inp["col_writes"]))
    pr["wn"] = _prep_relation(np.asarray(inp["row_written"]), np.asarray(inp["col_written"]))

    xa = np.asarray(inp["x_author"], dtype=F32)
    xp = np.asarray(inp["x_paper"], dtype=F32)
    if USE_FP8:
        # fp8 gather tables, viewed as bf16 [N, 128] for the byte-moving gather
        pr["xa8"] = xa.astype(FP8).view(np.uint16).view(BF16)
        pr["xp8"] = xp.astype(FP8).view(np.uint16).view(BF16)
    else:
        pr["xa8"] = xa.astype(BF16)
        pr["xp8"] = xp.astype(BF16)

    # per-core transposed x slices (for the self path of the dst shard)
    xta, xtp = [], []
    for c in range(NCORES):
        r0, r1 = c * NPAD, min(N, (c + 1) * NPAD)
        sa = np.zeros((D, NPAD), dtype=BF16)
        sp = np.zeros((D, NPAD), dtype=BF16)
        sa[:, : r1 - r0] = xa[r0:r1].T
        sp[:, : r1 - r0] = xp[r0:r1].T
        xta.append(sa)
        xtp.append(sp)
    pr["xta"], pr["xtp"] = xta, xtp

    W_sem = np.asarray(inp["W_sem"], dtype=F32)
    b_sem = np.asarray(inp["b_sem"], dtype=F32)
    w_score = np.asarray(inp["w_score"], dtype=F32)

    def w(name):
        return np.asarray(inp[name], dtype=F32)

    # folded weights per relation: (dst self weight, rel weight)
    for tag, wself, bself, wrel in (
        ("wr", w("W_self_paper"), w("b_self_paper"), w("W_rel_writes")),
        ("wn", w("W_self_author"), w("b_self_author"), w("W_rel_written")),
    ):
        pr[f"wp_self_{tag}"] = (0.5 * wself).astype(BF16)
        pr[f"wp_rel_{tag}"] = (0.5 * wrel).astype(BF16)
        pr[f"wq_rel_{tag}"] = (-0.5 * wrel).astype(BF16)
        pr[f"wf_self_{tag}"] = (wself @ W_sem).astype(BF16)
        pr[f"wf_rel_{tag}"] = (wrel @ W_sem).astype(BF16)
        # bias rows: [1, 3*D] = (0.5*b_self | b_self@W_sem + b_sem | b_sem)
        pr[f"brows_{tag}"] = np.concatenate([
            0.5 * bself, bself @ W_sem + b_sem, b_sem,
        ]).reshape(1, 3 * D).astype(BF16)

    # pre-scaled by the 0.5 from sigmoid(x) = 0.5*(1+tanh(x/2))
    pr["wrep"] = np.tile(0.5 * w_score, (P, 1)).astype(F32)
    pr["iota"] = np.tile(np.arange(P, dtype=F32), (P, 1)).astype(BF16)
    pr["ident"] = np.eye(P, dtype=F32).astype(BF16)
    pr["ones"] = np.ones((1, P), dtype=BF16)
    return pr


# ---------------------------------------------------------------- program
def build_program(nwin, c_lo_wr, c_hi_wr, c_lo_wn, c_hi_wn, scale=1,
                  nq=4, use_fp8=True, skip_gather=False, gather_only=False,
                  group=None, single_packet=False, tiny_idx=False,
                  gbufs=3, reload_consts=True, stream_consts=False):
    f32 = mybir.dt.float32
    bf16 = mybir.dt.bfloat16
    f8 = mybir.dt.float8e4 if use_fp8 else mybir.dt.bfloat16
    i16 = mybir.dt.int16
    AF = mybir.ActivationFunctionType
    OP = mybir.AluOpType

    npad = nwin * P

    g_ = GROUP if group is None else group

    def pairs_of(n):
        out = []
        w = 0
        while w < n:
            out.append(tuple(range(w, min(w + g_, n))))
            w += g_
        return out

    wpairs = pairs_of(nwin)

    def rel_geom(c_lo, c_hi):
        call_w = [c_lo[w] + c_hi[w] for w in range(nwin)]
        off = [0]
        for w in range(nwin):
            off.append(off[-1] + call_w[w])
        total_call = off[-1]
        # pair-ordered idx offsets (in 16-wrapped columns, x8 replicas)
        pinfo = []
        pos = 0
        for pr_ in wpairs:
            wlo = sum(c_lo[w] for w in pr_)
            whi = sum(c_hi[w] for w in pr_)
            pinfo.append((pos, wlo, pos + 8 * wlo, whi))
            pos += 8 * (wlo + whi)
        return dict(call_w=call_w, off=off, total_call=total_call,
                    pinfo=pinfo, total8=pos,
                    maxlo=max(i[1] for i in pinfo),
                    maxhi=max(i[3] for i in pinfo),
                    maxcall=max(call_w))

    geom_wr = rel_geom(c_lo_wr, c_hi_wr)
    geom_wn = rel_geom(c_lo_wn, c_hi_wn)

    nc = bacc.Bacc("TRN2", target_bir_lowering=False, debug=False,
                   num_swdge_queues=nq)

    TW = P if use_fp8 else D
    xa8 = nc.dram_tensor("xa8", [N, TW], bf16, kind="ExternalInput")
    xp8 = nc.dram_tensor("xp8", [N, TW], bf16, kind="ExternalInput")
    xta = nc.dram_tensor("xta", [D, npad], bf16, kind="ExternalInput")
    xtp = nc.dram_tensor("xtp", [D, npad], bf16, kind="ExternalInput")

    wnames = []
    for tag in ("wr", "wn"):
        wnames += [f"wp_self_{tag}", f"wp_rel_{tag}", f"wq_rel_{tag}",
                   f"wf_self_{tag}", f"wf_rel_{tag}"]
    wdram = {n: nc.dram_tensor(n, [D, D], bf16, kind="ExternalInput") for n in wnames}
    brow_d = {tag: nc.dram_tensor(f"brows_{tag}", [1, 3 * D], bf16,
                                  kind="ExternalInput") for tag in ("wr", "wn")}
    wrep_d = nc.dram_tensor("wrep", [P, D], f32, kind="ExternalInput")
    iota_d = nc.dram_tensor("iota", [P, P], bf16, kind="ExternalInput")
    ident_d = nc.dram_tensor("ident", [P, P], bf16, kind="ExternalInput")
    ones_d = nc.dram_tensor("ones", [1, P], bf16, kind="ExternalInput")

    idx_wr_d = nc.dram_tensor("idx_wr", [P, geom_wr["total8"]], i16, kind="ExternalInput")
    idx_wn_d = nc.dram_tensor("idx_wn", [P, geom_wn["total8"]], i16, kind="ExternalInput")
    colf_wr_d = nc.dram_tensor("colf_wr", [P, geom_wr["total_call"]], bf16, kind="ExternalInput")
    colf_wn_d = nc.dram_tensor("colf_wn", [P, geom_wn["total_call"]], bf16, kind="ExternalInput")
    recip_wr_d = nc.dram_tensor("recip_wr", [P, nwin], f32, kind="ExternalInput")
    recip_wn_d = nc.dram_tensor("recip_wn", [P, nwin], f32, kind="ExternalInput")

    oa = nc.dram_tensor("oa", [npad, D], bf16, kind="ExternalOutput")
    op_ = nc.dram_tensor("op", [npad, D], bf16, kind="ExternalOutput")

    with tile.TileContext(nc) as tc:
        with tc.tile_pool(name="const", bufs=1) as cpool, \
             tc.tile_pool(name="strm", bufs=2) as stpool, \
             tc.tile_pool(name="gbuf", bufs=gbufs) as gpool, \
             tc.tile_pool(name="oh", bufs=3) as ohpool, \
             tc.tile_pool(name="sb", bufs=3) as sbpool, \
             tc.tile_pool(name="mps", bufs=2, space="PSUM") as mpool, \
             tc.tile_pool(name="tps", bufs=1, space="PSUM") as tpool, \
             tc.tile_pool(name="dps", bufs=1, space="PSUM") as dpool:

            def load(dram, shape, dtype, tag, pool=None):
                if pool is stpool and not stream_consts:
                    pool = cpool
                t = (pool or cpool).tile(shape, dtype, tag=tag)
                nc.sync.dma_start(t[:], dram)
                return t

            def load_consts():
                """(Re)load all constants; per scale-iteration so the
                scale-unrolled timing program repeats the full pipeline.
                idx/colf first: they gate the next iteration's gathers,
                which are the kernel's critical path."""
                idx_wr_t = load(idx_wr_d[:], [P, geom_wr["total8"]], i16, "c_idxwr",
                                pool=stpool)
                idx_wn_t = load(idx_wn_d[:], [P, geom_wn["total8"]], i16, "c_idxwn",
                                pool=stpool)
                colf_wr_t = load(colf_wr_d[:], [P, geom_wr["total_call"]], bf16,
                                 "c_colfwr", pool=stpool)
                colf_wn_t = load(colf_wn_d[:], [P, geom_wn["total_call"]], bf16,
                                 "c_colfwn", pool=stpool)
                recip_wr_t = load(recip_wr_d[:], [P, nwin], f32, "c_recipwr",
                                  pool=stpool)
                recip_wn_t = load(recip_wn_d[:], [P, nwin], f32, "c_recipwn",
                                  pool=stpool)
                iota_t = load(iota_d[:], [P, P], bf16, "c_iota")
                ident_t = load(ident_d[:], [P, P], bf16, "c_ident")
                ones_t = load(ones_d[:], [1, P], bf16, "c_ones")
                wrep_t = load(wrep_d[:], [P, D], f32, "c_wrep")
                wt = {n: (load(wdram[n][0:P, :], [P, D], bf16, f"c_{n}0"),
                          load(wdram[n][P:D, :], [P, D], bf16, f"c_{n}1"))
                      for n in wnames}
                brow = {tag: load(brow_d[tag][:], [1, 3 * D], bf16, f"c_br{tag}")
                        for tag in ("wr", "wn")}
                xta_t = (load(xta[0:P, :], [P, npad], bf16, "c_xta0"),
                         load(xta[P:D, :], [P, npad], bf16, "c_xta1"))
                xtp_t = (load(xtp[0:P, :], [P, npad], bf16, "c_xtp0"),
                         load(xtp[P:D, :], [P, npad], bf16, "c_xtp1"))

                rels = [
                    dict(tag="wr", table=xa8, idx=idx_wr_t, colf=colf_wr_t,
                         recip=recip_wr_t, c_lo=c_lo_wr, c_hi=c_hi_wr,
                         geom=geom_wr, xt=xtp_t, q0=0, out=op_),
                    dict(tag="wn", table=xp8, idx=idx_wn_t, colf=colf_wn_t,
                         recip=recip_wn_t, c_lo=c_lo_wn, c_hi=c_hi_wn,
                         geom=geom_wn, xt=xta_t, q0=2, out=oa),
                ]
                for r in rels:
                    tag = r["tag"]
                    r["wp_self"] = wt[f"wp_self_{tag}"]
                    r["wp_rel"] = wt[f"wp_rel_{tag}"]
                    r["wq_rel"] = wt[f"wq_rel_{tag}"]
                    r["wf_self"] = wt[f"wf_self_{tag}"]
                    r["wf_rel"] = wt[f"wf_rel_{tag}"]
                    r["brow"] = brow[tag]
                return iota_t, ident_t, ones_t, wrep_t, rels

            def emit_pair_gather(ip, r):
                geom = r["geom"]
                io_lo, wlo, io_hi, whi = geom["pinfo"][ip]
                tag = r["tag"]
                g_lo = gpool.tile([P, geom["maxlo"], D], f8, tag=f"glo{tag}")
                g_hi = gpool.tile([P, geom["maxhi"], D], f8, tag=f"ghi{tag}")
                if not skip_gather:
                    n_lo, n_hi = wlo * P, whi * P
                    sl_lo, sl_hi = wlo, whi
                    if tiny_idx:
                        n_lo = n_hi = P
                        sl_lo = sl_hi = 1
                    out_lo = (g_lo.bitcast(bf16) if use_fp8 else g_lo)[:, 0:sl_lo, :]
                    nc.gpsimd.dma_gather(
                        out_lo, r["table"][:],
                        r["idx"][:, io_lo: io_lo + 8 * sl_lo],
                        n_lo, n_lo, TW, single_packet=single_packet,
                        queue_num=r["q0"] % nq)
                    out_hi = (g_hi.bitcast(bf16) if use_fp8 else g_hi)[:, 0:sl_hi, :]
                    nc.gpsimd.dma_gather(
                        out_hi, r["table"][HALF:, :],
                        r["idx"][:, io_hi: io_hi + 8 * sl_hi],
                        n_hi, n_hi, TW, single_packet=single_packet,
                        queue_num=(r["q0"] + 1) % nq)
                return g_lo, g_hi

            def emit_window(w, r, g_lo, g_hi, lo0, hi0):
                tag = r["tag"]
                geom = r["geom"]
                c_lo, c_hi = r["c_lo"][w], r["c_hi"][w]
                call = c_lo + c_hi
                co = geom["off"][w]

                oh = ohpool.tile([P, geom["maxcall"], P], f8, tag=f"oh{tag}")
                nc.vector.tensor_tensor(
                    out=oh[:, 0:call, :],
                    in0=r["colf"][:, co: co + call, None].to_broadcast([P, call, P]),
                    in1=iota_t[:, None, :].to_broadcast([P, call, P]),
                    op=OP.is_equal)

                m_ps = mpool.tile([P, D], f32, tag="m")
                for k in range(call):
                    rhs = (g_lo[:, lo0 + k, :] if k < c_lo
                           else g_hi[:, hi0 + k - c_lo, :])
                    nc.tensor.matmul(out=m_ps[:], lhsT=oh[:, k, :], rhs=rhs,
                                     start=(k == 0), stop=(k == call - 1))

                # deg-normalize on the scalar engine (per-dst 1/deg scale)
                m_sb = sbpool.tile([P, D], bf16, tag="m_sb")
                nc.scalar.activation(out=m_sb[:], in_=m_ps[:], func=AF.Copy,
                                     scale=r["recip"][:, w: w + 1])

                mt = []
                for h2 in range(2):
                    t_ps = tpool.tile([P, P], bf16, tag=f"t{h2}")
                    nc.tensor.transpose(out=t_ps[:],
                                        in_=m_sb[:, h2 * P: (h2 + 1) * P],
                                        identity=ident_t[:])
                    mt_sb = sbpool.tile([P, P], bf16, tag=f"mt{h2}")
                    nc.scalar.activation(out=mt_sb[:], in_=t_ps[:], func=AF.Copy)
                    mt.append(mt_sb)

                xsl0 = r["xt"][0][:, w * P: (w + 1) * P]
                xsl1 = r["xt"][1][:, w * P: (w + 1) * P]
                br = r["brow"]

                def dense(ps, parts, brow_slice):
                    for i, (lhsT, rhs) in enumerate(parts):
                        nc.tensor.matmul(out=ps, lhsT=lhsT, rhs=rhs,
                                         start=(i == 0), stop=False)
                    nc.tensor.matmul(out=ps, lhsT=ones_t[:], rhs=brow_slice,
                                     start=False, stop=True)
                    return ps

                pt = dpool.tile([P, D], f32, tag="p")
                qt = dpool.tile([P, D], f32, tag="q")
                zht = dpool.tile([P, D], f32, tag="zh")
                zat = dpool.tile([P, D], f32, tag="za")
                p_ps = dense(pt[:],
                             [(xsl0, r["wp_self"][0][:]), (xsl1, r["wp_self"][1][:]),
                              (mt[0][:], r["wp_rel"][0][:]), (mt[1][:], r["wp_rel"][1][:])],
                             br[:, 0:D])
                q_ps = dense(qt[:],
                             [(xsl0, r["wp_self"][0][:]), (xsl1, r["wp_self"][1][:]),
                              (mt[0][:], r["wq_rel"][0][:]), (mt[1][:], r["wq_rel"][1][:])],
                             br[:, 0:D])
                zh_ps = dense(zht[:],
                              [(xsl0, r["wf_self"][0][:]), (xsl1, r["wf_self"][1][:])],
                              br[:, D:2 * D])
                za_ps = dense(zat[:],
                              [(mt[0][:], r["wf_rel"][0][:]), (mt[1][:], r["wf_rel"][1][:])],
                              br[:, 2 * D:3 * D])

                th = sbpool.tile([P, D], f32, tag="th")
                nc.scalar.activation(out=th[:], in_=zh_ps, func=AF.Tanh)
                ta = sbpool.tile([P, D], f32, tag="ta")
                nc.scalar.activation(out=ta[:], in_=za_ps, func=AF.Tanh)

                v = sbpool.tile([P, D], f32, tag="v")
                nc.vector.tensor_tensor(out=v[:], in0=th[:], in1=ta[:],
                                        op=OP.subtract)
                vw = sbpool.tile([P, D], f32, tag="vw")
                nc.vector.tensor_tensor(out=vw[:], in0=v[:], in1=wrep_t[:],
                                        op=OP.mult)
                dsc = sbpool.tile([P, 1], f32, tag="dsc")
                nc.vector.tensor_reduce(out=dsc[:], in_=vw[:],
                                        axis=mybir.AxisListType.X, op=OP.add)

                t_sc = sbpool.tile([P, 1], f32, tag="tsc")
                nc.scalar.activation(out=t_sc[:], in_=dsc[:], func=AF.Tanh)

                wq = sbpool.tile([P, D], f32, tag="wq")
                nc.vector.tensor_scalar(out=wq[:], in0=q_ps,
                                        scalar1=t_sc[:, 0:1], scalar2=None,
                                        op0=OP.mult)
                outt = sbpool.tile([P, D], bf16, tag="outt")
                nc.vector.tensor_tensor(out=outt[:], in0=wq[:], in1=p_ps,
                                        op=OP.add)
                nc.sync.dma_start(r["out"][w * P: (w + 1) * P, :], outt[:])

            last_g = None
            for _s in range(scale):
                if _s == 0 or reload_consts:
                    iota_t, ident_t, ones_t, wrep_t, rels = load_consts()
                for ip, pr_ in enumerate(wpairs):
                    gt = {r["tag"]: emit_pair_gather(ip, r) for r in rels}
                    last_g = gt[rels[0]["tag"]][0]
                    if gather_only:
                        continue
                    for j, w in enumerate(pr_):
                        for r in rels:
                            g_lo, g_hi = gt[r["tag"]]
                            lo0 = sum(r["c_lo"][v] for v in pr_[:j])
                            hi0 = sum(r["c_hi"][v] for v in pr_[:j])
                            emit_window(w, r, g_lo, g_hi, lo0, hi0)
            if gather_only:
                fin = sbpool.tile([P, D], bf16, tag="fin")
                nc.vector.tensor_copy(out=fin[:], in_=last_g[:, 0, :])
                nc.sync.dma_start(oa[0:P, :], fin[:])

    nc.compile()
    return nc


# ---------------------------------------------------------------- driver
_PROG_CACHE = {}


def _get_program(key):
    if key not in _PROG_CACHE:
        _PROG_CACHE[key] = build_program(*key)
    return _PROG_CACHE[key]


def _make_in_maps(pr):
    shared = dict(
        xa8=pr["xa8"], xp8=pr["xp8"],
        iota=pr["iota"], ident=pr["ident"], ones=pr["ones"], wrep=pr["wrep"],
        brows_wr=pr["brows_wr"], brows_wn=pr["brows_wn"],
    )
    for tag in ("wr", "wn"):
        for nm in ("wp_self", "wp_rel", "wq_rel", "wf_self", "wf_rel"):
            shared[f"{nm}_{tag}"] = pr[f"{nm}_{tag}"]
    idx_wr, colf_wr, recip_wr, _, _ = pr["wr"]
    idx_wn, colf_wn, recip_wn, _, _ = pr["wn"]
    in_maps = []
    for c in range(NCORES):
        w0, w1 = c * NWIN, (c + 1) * NWIN
        m = dict(shared)
        m["xta"] = pr["xta"][c]
        m["xtp"] = pr["xtp"][c]
        m["idx_wr"] = np.ascontiguousarray(np.tile(idx_wr[c], (8, 1)))
        m["idx_wn"] = np.ascontiguousarray(np.tile(idx_wn[c], (8, 1)))
        m["colf_wr"] = np.ascontiguousarray(colf_wr[c]).astype(BF16)
        m["colf_wn"] = np.ascontiguousarray(colf_wn[c]).astype(BF16)
        m["recip_wr"] = np.ascontiguousarray(recip_wr[:, w0:w1])
        m["recip_wn"] = np.ascontiguousarray(recip_wn[:, w0:w1])
        in_maps.append(m)
    return in_maps


def run(trace=False, tmpdir=None, **inputs):
    pr = _host_prep(inputs)
    _, _, _, c_lo_wr, c_hi_wr = pr["wr"]
    _, _, _, c_lo_wn, c_hi_wn = pr["wn"]
    nc = _get_program((NWIN, c_lo_wr, c_hi_wr, c_lo_wn, c_hi_wn, 1, NQ, USE_FP8))
    in_maps = _make_in_maps(pr)
    res = run_bass_kernel_spmd(nc, in_maps, list(range(NCORES)),
                               trace=trace, tmpdir=tmpdir)
    oa = np.empty((N, D), dtype=F32)
    op = np.empty((N, D), dtype=F32)
    for c in range(NCORES):
        r0, r1 = c * NPAD, min(N, (c + 1) * NPAD)
        oa[r0:r1] = res.results[c]["oa"][: r1 - r0].astype(F32)
        op[r0:r1] = res.results[c]["op"][: r1 - r0].astype(F32)
    return (oa, op), res


def kernel(**inputs):
    (oa, op), _ = run(trace=False, **inputs)
    return (oa, op)



# revision 38
# speedup vs baseline: 7.9408x; 1.0308x over previous
"""HANConv Trainium2 kernel (8 NeuronCores, SPMD, full-I/O contract).

Strategy (v2)
-------------
Destination-sharded, fully core-independent:
  * Each core owns 1/8 of destination nodes for BOTH relations
    (writes: author->paper, written: paper->author).
  * Edges are sorted by (dst window, src half, src) on host. Per window,
    source rows are gathered as fp8(e4m3) 256B rows via gpsimd.dma_gather,
    round-robin over 4 SWDGE queues (4x the single-queue descriptor
    throughput; the gather is descriptor-bound, so fp8 also halves bytes),
    and segment-summed with fp8 one-hot matmuls accumulating in f32 PSUM.
  * Aggregating RAW features (M = A @ x, then per-dst 1/deg scale on the
    scalar engine) lets every later transform be a dense matmul from M with
    host-folded weights, so no cross-core exchange is ever needed.
  * 2-candidate semantic softmax is rewritten tanh-only:
        out = p + tanh(0.5*(s_h - s_agg)) * q
        p = 0.5*(h + agg),  q = 0.5*(h - agg)
    with the 0.5 factors folded into the weights on host. The scalar
    engine therefore never switches activation tables.
  * Scores use one fused DVE tensor_tensor_reduce:
        dsc = 0.5 * sum(w_score * (tanh(z_h) - tanh(z_agg)))
  * Self path computed from host-transposed x slices with folded weights.
  * Outputs written bf16 and upcast to f32 on host.
"""

import sys

sys.path.insert(0, "/opt/trn_rl_repo")

import numpy as np
import ml_dtypes

import concourse.bacc as bacc
import concourse.mybir as mybir
import concourse.tile as tile
from concourse.bass_utils import run_bass_kernel_spmd

P = 128
N = 50000
D = 256
HALF = 32768  # int16 gather index limit
NCORES = 8
NW_TOTAL = (N + P - 1) // P            # 391 destination windows
NWIN = (NW_TOTAL + NCORES - 1) // NCORES  # 49 windows per core
NW_ALLOC = NWIN * NCORES               # 392 (incl. 1 phantom window)
NPAD = NWIN * P                        # 6272 output rows per core

BF16 = ml_dtypes.bfloat16
FP8 = ml_dtypes.float8_e4m3
F32 = np.float32

USE_FP8 = True
NQ = 4


GROUP = 2


def _pairs(group=None):
    """Window slots grouped into gather groups of `group` windows."""
    g = GROUP if group is None else group
    out = []
    w = 0
    while w < NWIN:
        out.append(tuple(range(w, min(w + g, NWIN))))
        w += g
    return out


PAD_NEG = False  # -1 gather padding (trailing, per region) for dyn_counts


# ---------------------------------------------------------------- host prep
def _prep_relation(row, col):
    """Sort edges by (dst window, src half, src); per-slot dynamic widths.

    Slot widths c_lo/c_hi[w] are the max over the 8 cores so the SPMD
    program is common. Gathers are issued per window PAIR (lo and hi
    halves separately) so the idx layout per core is, in pair order:
      [pair lo: slots w0|w1 ...][pair hi: slots w0|w1 ...] ...
    colf layout per core is per-slot: [slot: lo blocks | hi blocks] ...

    Returns (idx16_percore [NCORES,16,total8], colf_percore
    [NCORES,P,total_call], recip [P,NW_ALLOC], c_lo[NWIN], c_hi[NWIN]).
    """
    E = row.shape[0]
    key = (col // P) * 2 + (row >= HALF)
    order = np.lexsort((row, key))
    ks = key[order]
    rs = row[order].astype(np.int64)
    cs = col[order].astype(np.int64)

    counts = np.bincount(key, minlength=NW_ALLOC * 2).astype(np.int64)
    lo_cnt = counts[0::2].reshape(NCORES, NWIN)
    hi_cnt = counts[1::2].reshape(NCORES, NWIN)
    c_lo = np.maximum(1, -(-lo_cnt.max(axis=0) // P))  # [NWIN]
    c_hi = np.maximum(1, -(-hi_cnt.max(axis=0) // P))  # [NWIN]
    call_w = c_lo + c_hi
    off = np.zeros(NWIN + 1, dtype=np.int64)
    off[1:] = np.cumsum(call_w)
    total_call = int(off[-1])

    # idx layout offsets (in index units) per (slot, half), pair-ordered
    idx_base = np.zeros((NWIN, 2), dtype=np.int64)
    pos = 0
    for pr_ in _pairs():
        for w in pr_:
            idx_base[w, 0] = pos
            pos += int(c_lo[w]) * P
        for w in pr_:
            idx_base[w, 1] = pos
            pos += int(c_hi[w]) * P
    total_idx = pos

    grp_start = np.zeros(NW_ALLOC * 2 + 1, dtype=np.int64)
    np.cumsum(counts, out=grp_start[1:])
    rank = np.arange(E, dtype=np.int64) - grp_start[ks]
    w_of = ks // 2
    core = w_of // NWIN
    slot = w_of % NWIN
    hi_of = ks % 2

    pad_val = -1 if PAD_NEG else 0
    idx_flat = np.full(NCORES * total_idx, pad_val, dtype=np.int16)
    ipos = core * total_idx + idx_base[slot, hi_of] + rank
    idx_flat[ipos] = (rs - HALF * hi_of).astype(np.int16)
    if PAD_NEG:
        # empty (core, slot, half) regions keep one valid idx (row 0) so
        # num_idxs_reg >= 1 holds
        for c_ in range(NCORES):
            for w_ in range(NWIN):
                if lo_cnt[c_, w_] == 0:
                    idx_flat[c_ * total_idx + idx_base[w_, 0]] = 0
                if hi_cnt[c_, w_] == 0:
                    idx_flat[c_ * total_idx + idx_base[w_, 1]] = 0
    col_flat = np.full(NCORES * total_call * P, -1.0, dtype=F32)
    cpos = core * (total_call * P) + (off[slot] + hi_of * c_lo[slot]) * P + rank
    col_flat[cpos] = (cs - w_of * P).astype(F32)

    # wrap idx per gather region: region r of length L -> [16, L*8/16...]
    idx_pc = idx_flat.reshape(NCORES, total_idx)
    parts = []
    pos = 0
    for pr_ in _pairs():
        for half, carr in ((0, c_lo), (1, c_hi)):
            L = int(sum(carr[w] for w in pr_)) * P
            reg = idx_pc[:, pos: pos + L]
            parts.append(reg.reshape(NCORES, L // 16, 16).transpose(0, 2, 1))
            pos += L
    idx16 = np.concatenate(parts, axis=2)  # [NCORES, 16, total_idx//16]

    colf = col_flat.reshape(NCORES, total_call, P).transpose(0, 2, 1)

    deg = np.bincount(col, minlength=NW_ALLOC * P).astype(F32)[: NW_ALLOC * P]
    recip = (1.0 / np.maximum(deg, 1.0)).reshape(NW_ALLOC, P).T  # [P, NW]
    cnts = np.stack([np.maximum(lo_cnt, 1), np.maximum(hi_cnt, 1)],
                    axis=2).astype(np.int32)  # [NCORES, NWIN, 2] true counts
    return idx16, colf, recip, tuple(int(x) for x in c_lo), tuple(int(x) for x in c_hi), cnts


def _host_prep(inp):
    pr = {}
    pr["wr"] = _prep_relation(np.asarray(inp["row_writes"]), np.asarray(inp["col_writes"]))
    pr["wn"] = _prep_relation(np.asarray(inp["row_written"]), np.asarray(inp["col_written"]))

    xa = np.asarray(inp["x_author"], dtype=F32)
    xp = np.asarray(inp["x_paper"], dtype=F32)
    if USE_FP8:
        # fp8 gather tables, viewed as bf16 [N, 128] for the byte-moving gather
        pr["xa8"] = xa.astype(FP8).view(np.uint16).view(BF16)
        pr["xp8"] = xp.astype(FP8).view(np.uint16).view(BF16)
    else:
        pr["xa8"] = xa.astype(BF16)
        pr["xp8"] = xp.astype(BF16)

    # per-core transposed x slices (for the self path of the dst shard)
    xta, xtp = [], []
    for c in range(NCORES):
        r0, r1 = c * NPAD, min(N, (c + 1) * NPAD)
        sa = np.zeros((D, NPAD), dtype=BF16)
        sp = np.zeros((D, NPAD), dtype=BF16)
        sa[:, : r1 - r0] = xa[r0:r1].T
        sp[:, : r1 - r0] = xp[r0:r1].T
        xta.append(sa)
        xtp.append(sp)
    pr["xta"], pr["xtp"] = xta, xtp

    W_sem = np.asarray(inp["W_sem"], dtype=F32)
    b_sem = np.asarray(inp["b_sem"], dtype=F32)
    w_score = np.asarray(inp["w_score"], dtype=F32)

    def w(name):
        return np.asarray(inp[name], dtype=F32)

    # folded weights per relation: (dst self weight, rel weight)
    for tag, wself, bself, wrel in (
        ("wr", w("W_self_paper"), w("b_self_paper"), w("W_rel_writes")),
        ("wn", w("W_self_author"), w("b_self_author"), w("W_rel_written")),
    ):
        pr[f"wp_self_{tag}"] = (0.5 * wself).astype(BF16)
        pr[f"wp_rel_{tag}"] = (0.5 * wrel).astype(BF16)
        pr[f"wq_rel_{tag}"] = (-0.5 * wrel).astype(BF16)
        pr[f"wf_self_{tag}"] = (wself @ W_sem).astype(BF16)
        pr[f"wf_rel_{tag}"] = (wrel @ W_sem).astype(BF16)
        # bias rows: [1, 3*D] = (0.5*b_self | b_self@W_sem + b_sem | b_sem)
        pr[f"brows_{tag}"] = np.concatenate([
            0.5 * bself, bself @ W_sem + b_sem, b_sem,
        ]).reshape(1, 3 * D).astype(BF16)

    # pre-scaled by the 0.5 from sigmoid(x) = 0.5*(1+tanh(x/2))
    pr["wrep"] = np.tile(0.5 * w_score, (P, 1)).astype(F32)
    pr["iota"] = np.tile(np.arange(P, dtype=F32), (P, 1)).astype(BF16)
    pr["ident"] = np.eye(P, dtype=F32).astype(BF16)
    pr["ones"] = np.ones((1, P), dtype=BF16)
    return pr


# ---------------------------------------------------------------- program
def build_program(nwin, c_lo_wr, c_hi_wr, c_lo_wn, c_hi_wn, scale=1,
                  nq=4, use_fp8=True, skip_gather=False, gather_only=False,
                  group=None, single_packet=False, tiny_idx=False,
                  gbufs=3, reload_consts=True, stream_consts=False,
                  sbufs=3, ohbufs=3, dyn_counts=False, stream_idx=False):
    f32 = mybir.dt.float32
    bf16 = mybir.dt.bfloat16
    f8 = mybir.dt.float8e4 if use_fp8 else mybir.dt.bfloat16
    i16 = mybir.dt.int16
    AF = mybir.ActivationFunctionType
    OP = mybir.AluOpType

    npad = nwin * P

    g_ = GROUP if group is None else group

    def pairs_of(n):
        out = []
        w = 0
        while w < n:
            out.append(tuple(range(w, min(w + g_, n))))
            w += g_
        return out

    wpairs = pairs_of(nwin)

    def rel_geom(c_lo, c_hi):
        call_w = [c_lo[w] + c_hi[w] for w in range(nwin)]
        off = [0]
        for w in range(nwin):
            off.append(off[-1] + call_w[w])
        total_call = off[-1]
        # pair-ordered idx offsets (in 16-wrapped columns, x8 replicas)
        pinfo = []
        pos = 0
        for pr_ in wpairs:
            wlo = sum(c_lo[w] for w in pr_)
            whi = sum(c_hi[w] for w in pr_)
            pinfo.append((pos, wlo, pos + 8 * wlo, whi))
            pos += 8 * (wlo + whi)
        return dict(call_w=call_w, off=off, total_call=total_call,
                    pinfo=pinfo, total8=pos,
                    maxlo=max(i[1] for i in pinfo),
                    maxhi=max(i[3] for i in pinfo),
                    maxcall=max(call_w))

    geom_wr = rel_geom(c_lo_wr, c_hi_wr)
    geom_wn = rel_geom(c_lo_wn, c_hi_wn)

    nc = bacc.Bacc("TRN2", target_bir_lowering=False, debug=False,
                   num_swdge_queues=nq)

    TW = P if use_fp8 else D
    xa8 = nc.dram_tensor("xa8", [N, TW], bf16, kind="ExternalInput")
    xp8 = nc.dram_tensor("xp8", [N, TW], bf16, kind="ExternalInput")
    xta = nc.dram_tensor("xta", [D, npad], bf16, kind="ExternalInput")
    xtp = nc.dram_tensor("xtp", [D, npad], bf16, kind="ExternalInput")

    wnames = []
    for tag in ("wr", "wn"):
        wnames += [f"wp_self_{tag}", f"wp_rel_{tag}", f"wq_rel_{tag}",
                   f"wf_self_{tag}", f"wf_rel_{tag}"]
    wdram = {n: nc.dram_tensor(n, [D, D], bf16, kind="ExternalInput") for n in wnames}
    brow_d = {tag: nc.dram_tensor(f"brows_{tag}", [1, 3 * D], bf16,
                                  kind="ExternalInput") for tag in ("wr", "wn")}
    wrep_d = nc.dram_tensor("wrep", [P, D], f32, kind="ExternalInput")
    iota_d = nc.dram_tensor("iota", [P, P], bf16, kind="ExternalInput")
    ident_d = nc.dram_tensor("ident", [P, P], bf16, kind="ExternalInput")
    ones_d = nc.dram_tensor("ones", [1, P], bf16, kind="ExternalInput")

    idx_wr_d = nc.dram_tensor("idx_wr", [P, geom_wr["total8"]], i16, kind="ExternalInput")
    idx_wn_d = nc.dram_tensor("idx_wn", [P, geom_wn["total8"]], i16, kind="ExternalInput")
    colf_wr_d = nc.dram_tensor("colf_wr", [P, geom_wr["total_call"]], bf16, kind="ExternalInput")
    colf_wn_d = nc.dram_tensor("colf_wn", [P, geom_wn["total_call"]], bf16, kind="ExternalInput")
    recip_wr_d = nc.dram_tensor("recip_wr", [P, nwin], f32, kind="ExternalInput")
    recip_wn_d = nc.dram_tensor("recip_wn", [P, nwin], f32, kind="ExternalInput")
    cnts_d = (nc.dram_tensor("cnts", [1, 4 * nwin], mybir.dt.int32,
                             kind="ExternalInput") if dyn_counts else None)

    oa = nc.dram_tensor("oa", [npad, D], bf16, kind="ExternalOutput")
    op_ = nc.dram_tensor("op", [npad, D], bf16, kind="ExternalOutput")

    with tile.TileContext(nc) as tc:
        with tc.tile_pool(name="const", bufs=1) as cpool, \
             tc.tile_pool(name="strm", bufs=2) as stpool, \
             tc.tile_pool(name="gbuf", bufs=gbufs) as gpool, \
             tc.tile_pool(name="oh", bufs=ohbufs) as ohpool, \
             tc.tile_pool(name="sb", bufs=sbufs) as sbpool, \
             tc.tile_pool(name="mps", bufs=2, space="PSUM") as mpool, \
             tc.tile_pool(name="tps", bufs=1, space="PSUM") as tpool, \
             tc.tile_pool(name="dps", bufs=1, space="PSUM") as dpool:

            def load(dram, shape, dtype, tag, pool=None):
                t = (pool or cpool).tile(shape, dtype, tag=tag)
                nc.sync.dma_start(t[:], dram)
                return t

            def load_consts():
                """(Re)load all constants; per scale-iteration so the
                scale-unrolled timing program repeats the full pipeline.
                idx/colf first: they gate the next iteration's gathers,
                which are the kernel's critical path."""
                ipool = stpool if (stream_idx or stream_consts) else None
                cfpool = stpool if stream_consts else None
                idx_wr_t = load(idx_wr_d[:], [P, geom_wr["total8"]], i16, "c_idxwr",
                                pool=ipool)
                idx_wn_t = load(idx_wn_d[:], [P, geom_wn["total8"]], i16, "c_idxwn",
                                pool=ipool)
                colf_wr_t = load(colf_wr_d[:], [P, geom_wr["total_call"]], bf16,
                                 "c_colfwr", pool=cfpool)
                colf_wn_t = load(colf_wn_d[:], [P, geom_wn["total_call"]], bf16,
                                 "c_colfwn", pool=cfpool)
                recip_wr_t = load(recip_wr_d[:], [P, nwin], f32, "c_recipwr",
                                  pool=cfpool)
                recip_wn_t = load(recip_wn_d[:], [P, nwin], f32, "c_recipwn",
                                  pool=cfpool)
                cnts_t = (load(cnts_d[:], [1, 4 * nwin], mybir.dt.int32,
                               "c_cnts") if dyn_counts else None)
                iota_t = load(iota_d[:], [P, P], bf16, "c_iota")
                ident_t = load(ident_d[:], [P, P], bf16, "c_ident")
                ones_t = load(ones_d[:], [1, P], bf16, "c_ones")
                wrep_t = load(wrep_d[:], [P, D], f32, "c_wrep")
                wt = {n: (load(wdram[n][0:P, :], [P, D], bf16, f"c_{n}0"),
                          load(wdram[n][P:D, :], [P, D], bf16, f"c_{n}1"))
                      for n in wnames}
                brow = {tag: load(brow_d[tag][:], [1, 3 * D], bf16, f"c_br{tag}")
                        for tag in ("wr", "wn")}
                xta_t = (load(xta[0:P, :], [P, npad], bf16, "c_xta0"),
                         load(xta[P:D, :], [P, npad], bf16, "c_xta1"))
                xtp_t = (load(xtp[0:P, :], [P, npad], bf16, "c_xtp0"),
                         load(xtp[P:D, :], [P, npad], bf16, "c_xtp1"))

                rels = [
                    dict(tag="wr", ci=0, table=xa8, idx=idx_wr_t, colf=colf_wr_t,
                         recip=recip_wr_t, c_lo=c_lo_wr, c_hi=c_hi_wr,
                         geom=geom_wr, xt=xtp_t, q0=0, out=op_),
                    dict(tag="wn", ci=1, table=xp8, idx=idx_wn_t, colf=colf_wn_t,
                         recip=recip_wn_t, c_lo=c_lo_wn, c_hi=c_hi_wn,
                         geom=geom_wn, xt=xta_t, q0=2, out=oa),
                ]
                for r in rels:
                    tag = r["tag"]
                    r["wp_self"] = wt[f"wp_self_{tag}"]
                    r["wp_rel"] = wt[f"wp_rel_{tag}"]
                    r["wq_rel"] = wt[f"wq_rel_{tag}"]
                    r["wf_self"] = wt[f"wf_self_{tag}"]
                    r["wf_rel"] = wt[f"wf_rel_{tag}"]
                    r["brow"] = brow[tag]
                return iota_t, ident_t, ones_t, wrep_t, rels, cnts_t

            def emit_pair_gather(ip, r):
                geom = r["geom"]
                io_lo, wlo, io_hi, whi = geom["pinfo"][ip]
                tag = r["tag"]
                g_lo = gpool.tile([P, geom["maxlo"], D], f8, tag=f"glo{tag}")
                g_hi = gpool.tile([P, geom["maxhi"], D], f8, tag=f"ghi{tag}")
                if not skip_gather:
                    n_lo, n_hi = wlo * P, whi * P
                    sl_lo, sl_hi = wlo, whi
                    if tiny_idx:
                        n_lo = n_hi = P
                        sl_lo = sl_hi = 1
                    reg_lo, reg_hi = n_lo, n_hi
                    if dyn_counts:
                        # dyn_counts requires group=1: ip == window index
                        cb = 4 * ip + 2 * r["ci"]
                        reg_lo = nc.values_load(
                            cnts_t[0:1, cb: cb + 1],
                            engines=(mybir.EngineType.Pool,),
                            min_val=1, max_val=n_lo,
                            skip_runtime_bounds_check=True)
                        reg_hi = nc.values_load(
                            cnts_t[0:1, cb + 1: cb + 2],
                            engines=(mybir.EngineType.Pool,),
                            min_val=1, max_val=n_hi,
                            skip_runtime_bounds_check=True)
                    out_lo = (g_lo.bitcast(bf16) if use_fp8 else g_lo)[:, 0:sl_lo, :]
                    nc.gpsimd.dma_gather(
                        out_lo, r["table"][:],
                        r["idx"][:, io_lo: io_lo + 8 * sl_lo],
                        n_lo, reg_lo, TW, single_packet=single_packet,
                        queue_num=r["q0"] % nq)
                    out_hi = (g_hi.bitcast(bf16) if use_fp8 else g_hi)[:, 0:sl_hi, :]
                    nc.gpsimd.dma_gather(
                        out_hi, r["table"][HALF:, :],
                        r["idx"][:, io_hi: io_hi + 8 * sl_hi],
                        n_hi, reg_hi, TW, single_packet=single_packet,
                        queue_num=(r["q0"] + 1) % nq)
                return g_lo, g_hi

            def emit_window(w, r, g_lo, g_hi, lo0, hi0):
                tag = r["tag"]
                geom = r["geom"]
                c_lo, c_hi = r["c_lo"][w], r["c_hi"][w]
                call = c_lo + c_hi
                co = geom["off"][w]

                oh = ohpool.tile([P, geom["maxcall"], P], f8, tag=f"oh{tag}")
                nc.vector.tensor_tensor(
                    out=oh[:, 0:call, :],
                    in0=r["colf"][:, co: co + call, None].to_broadcast([P, call, P]),
                    in1=iota_t[:, None, :].to_broadcast([P, call, P]),
                    op=OP.is_equal)

                m_ps = mpool.tile([P, D], f32, tag="m")
                for k in range(call):
                    rhs = (g_lo[:, lo0 + k, :] if k < c_lo
                           else g_hi[:, hi0 + k - c_lo, :])
                    nc.tensor.matmul(out=m_ps[:], lhsT=oh[:, k, :], rhs=rhs,
                                     start=(k == 0), stop=(k == call - 1))

                # deg-normalize on the scalar engine (per-dst 1/deg scale)
                m_sb = sbpool.tile([P, D], bf16, tag="m_sb")
                nc.scalar.activation(out=m_sb[:], in_=m_ps[:], func=AF.Copy,
                                     scale=r["recip"][:, w: w + 1])

                mt = []
                for h2 in range(2):
                    t_ps = tpool.tile([P, P], bf16, tag=f"t{h2}")
                    nc.tensor.transpose(out=t_ps[:],
                                        in_=m_sb[:, h2 * P: (h2 + 1) * P],
                                        identity=ident_t[:])
                    mt_sb = sbpool.tile([P, P], bf16, tag=f"mt{h2}")
                    nc.scalar.activation(out=mt_sb[:], in_=t_ps[:], func=AF.Copy)
                    mt.append(mt_sb)

                xsl0 = r["xt"][0][:, w * P: (w + 1) * P]
                xsl1 = r["xt"][1][:, w * P: (w + 1) * P]
                br = r["brow"]

                def dense(ps, parts, brow_slice):
                    for i, (lhsT, rhs) in enumerate(parts):
                        nc.tensor.matmul(out=ps, lhsT=lhsT, rhs=rhs,
                                         start=(i == 0), stop=False)
                    nc.tensor.matmul(out=ps, lhsT=ones_t[:], rhs=brow_slice,
                                     start=False, stop=True)
                    return ps

                pt = dpool.tile([P, D], f32, tag="p")
                qt = dpool.tile([P, D], f32, tag="q")
                zht = dpool.tile([P, D], f32, tag="zh")
                zat = dpool.tile([P, D], f32, tag="za")
                p_ps = dense(pt[:],
                             [(xsl0, r["wp_self"][0][:]), (xsl1, r["wp_self"][1][:]),
                              (mt[0][:], r["wp_rel"][0][:]), (mt[1][:], r["wp_rel"][1][:])],
                             br[:, 0:D])
                q_ps = dense(qt[:],
                             [(xsl0, r["wp_self"][0][:]), (xsl1, r["wp_self"][1][:]),
                              (mt[0][:], r["wq_rel"][0][:]), (mt[1][:], r["wq_rel"][1][:])],
                             br[:, 0:D])
                zh_ps = dense(zht[:],
                              [(xsl0, r["wf_self"][0][:]), (xsl1, r["wf_self"][1][:])],
                              br[:, D:2 * D])
                za_ps = dense(zat[:],
                              [(mt[0][:], r["wf_rel"][0][:]), (mt[1][:], r["wf_rel"][1][:])],
                              br[:, 2 * D:3 * D])

                th = sbpool.tile([P, D], f32, tag="th")
                nc.scalar.activation(out=th[:], in_=zh_ps, func=AF.Tanh)
                ta = sbpool.tile([P, D], f32, tag="ta")
                nc.scalar.activation(out=ta[:], in_=za_ps, func=AF.Tanh)

                v = sbpool.tile([P, D], f32, tag="v")
                nc.vector.tensor_tensor(out=v[:], in0=th[:], in1=ta[:],
                                        op=OP.subtract)
                vw = sbpool.tile([P, D], f32, tag="vw")
                nc.vector.tensor_tensor(out=vw[:], in0=v[:], in1=wrep_t[:],
                                        op=OP.mult)
                dsc = sbpool.tile([P, 1], f32, tag="dsc")
                nc.vector.tensor_reduce(out=dsc[:], in_=vw[:],
                                        axis=mybir.AxisListType.X, op=OP.add)

                t_sc = sbpool.tile([P, 1], f32, tag="tsc")
                nc.scalar.activation(out=t_sc[:], in_=dsc[:], func=AF.Tanh)

                wq = sbpool.tile([P, D], f32, tag="wq")
                nc.vector.tensor_scalar(out=wq[:], in0=q_ps,
                                        scalar1=t_sc[:, 0:1], scalar2=None,
                                        op0=OP.mult)
                outt = sbpool.tile([P, D], bf16, tag="outt")
                nc.vector.tensor_tensor(out=outt[:], in0=wq[:], in1=p_ps,
                                        op=OP.add)
                nc.sync.dma_start(r["out"][w * P: (w + 1) * P, :], outt[:])

            last_g = None
            if dyn_counts:
                assert g_ == 1, "dyn_counts needs group=1 (trailing-only pad)"
                # memset every gather buffer once: skipped (padded) rows leave
                # stale SBUF bytes; fp8 NaN bit patterns would poison the
                # one-hot matmuls (NaN * 0 = NaN)
                for _b in range(gbufs):
                    for tg in ("wr", "wn"):
                        gmx = geom_wr if tg == "wr" else geom_wn
                        gl = gpool.tile([P, gmx["maxlo"], D], f8, tag=f"glo{tg}")
                        nc.vector.memset(gl.bitcast(bf16)[:], 0.0)
                        gh = gpool.tile([P, gmx["maxhi"], D], f8, tag=f"ghi{tg}")
                        nc.vector.memset(gh.bitcast(bf16)[:], 0.0)
            for _s in range(scale):
                if _s == 0 or reload_consts:
                    iota_t, ident_t, ones_t, wrep_t, rels, cnts_t = load_consts()
                for ip, pr_ in enumerate(wpairs):
                    gt = {r["tag"]: emit_pair_gather(ip, r) for r in rels}
                    last_g = gt[rels[0]["tag"]][0]
                    if gather_only:
                        continue
                    for j, w in enumerate(pr_):
                        for r in rels:
                            g_lo, g_hi = gt[r["tag"]]
                            lo0 = sum(r["c_lo"][v] for v in pr_[:j])
                            hi0 = sum(r["c_hi"][v] for v in pr_[:j])
                            emit_window(w, r, g_lo, g_hi, lo0, hi0)
            if gather_only:
                fin = sbpool.tile([P, D], bf16, tag="fin")
                nc.vector.tensor_copy(out=fin[:], in_=last_g[:, 0, :])
                nc.sync.dma_start(oa[0:P, :], fin[:])

    nc.compile()
    return nc


# ---------------------------------------------------------------- driver
_PROG_CACHE = {}


def _get_program(key):
    if key not in _PROG_CACHE:
        _PROG_CACHE[key] = build_program(*key)
    return _PROG_CACHE[key]


def _make_in_maps(pr):
    shared = dict(
        xa8=pr["xa8"], xp8=pr["xp8"],
        iota=pr["iota"], ident=pr["ident"], ones=pr["ones"], wrep=pr["wrep"],
        brows_wr=pr["brows_wr"], brows_wn=pr["brows_wn"],
    )
    for tag in ("wr", "wn"):
        for nm in ("wp_self", "wp_rel", "wq_rel", "wf_self", "wf_rel"):
            shared[f"{nm}_{tag}"] = pr[f"{nm}_{tag}"]
    idx_wr, colf_wr, recip_wr, _, _, cnts_wr = pr["wr"]
    idx_wn, colf_wn, recip_wn, _, _, cnts_wn = pr["wn"]
    in_maps = []
    for c in range(NCORES):
        w0, w1 = c * NWIN, (c + 1) * NWIN
        m = dict(shared)
        m["xta"] = pr["xta"][c]
        m["xtp"] = pr["xtp"][c]
        m["idx_wr"] = np.ascontiguousarray(np.tile(idx_wr[c], (8, 1)))
        m["idx_wn"] = np.ascontiguousarray(np.tile(idx_wn[c], (8, 1)))
        m["colf_wr"] = np.ascontiguousarray(colf_wr[c]).astype(BF16)
        m["colf_wn"] = np.ascontiguousarray(colf_wn[c]).astype(BF16)
        m["recip_wr"] = np.ascontiguousarray(recip_wr[:, w0:w1])
        m["recip_wn"] = np.ascontiguousarray(recip_wn[:, w0:w1])
        # per-call true gather counts: per window (wr-lo, wr-hi, wn-lo, wn-hi)
        m["cnts"] = np.ascontiguousarray(
            np.concatenate([cnts_wr[c], cnts_wn[c]], axis=1)
            .reshape(1, 4 * NWIN))
        in_maps.append(m)
    return in_maps


def run(trace=False, tmpdir=None, **inputs):
    pr = _host_prep(inputs)
    _, _, _, c_lo_wr, c_hi_wr, _ = pr["wr"]
    _, _, _, c_lo_wn, c_hi_wn, _ = pr["wn"]
    nc = _get_program((NWIN, c_lo_wr, c_hi_wr, c_lo_wn, c_hi_wn, 1, NQ, USE_FP8))
    in_maps = _make_in_maps(pr)
    res = run_bass_kernel_spmd(nc, in_maps, list(range(NCORES)),
                               trace=trace, tmpdir=tmpdir)
    oa = np.empty((N, D), dtype=F32)
    op = np.empty((N, D), dtype=F32)
    for c in range(NCORES):
        r0, r1 = c * NPAD, min(N, (c + 1) * NPAD)
        oa[r0:r1] = res.results[c]["oa"][: r1 - r0].astype(F32)
        op[r0:r1] = res.results[c]["op"][: r1 - r0].astype(F32)
    return (oa, op), res


def kernel(**inputs):
    (oa, op), _ = run(trace=False, **inputs)
    return (oa, op)

